# revision 12
# baseline (speedup 1.0000x reference)
"""Bass/Trainium2 kernel for nn_BigramLanguageModel (6-layer GPT, B=8,T=1024,C=768,V=32000).

Strategy: data-parallel over batch across the 8 NeuronCores (one batch element
per core, no collectives). Each core runs the full transformer + lm_head +
logsumexp for its sequence; the host folds LN weights/biases into adjacent
matmul weights, gathers embeddings, and combines per-core loss partials.
"""

import os
import numpy as np
import ml_dtypes

import concourse.bass as bass
import concourse.mybir as mybir
import concourse.tile as tile
from concourse import bacc
from concourse.bass_utils import run_bass_kernel_spmd

F32 = mybir.dt.float32
BF16 = mybir.dt.bfloat16
AF = mybir.ActivationFunctionType
ALU = mybir.AluOpType
AX = mybir.AxisListType

V = 32000
T = 1024
C = 768
L = 6
H = 8
B = 8
D = 96          # real head dim
DP = 128        # padded head dim
CP = H * DP     # 1024, padded attn-concat dim
F = 4 * C       # 3072
EPS = 1e-5
NCORES = 8
NT = T // 128   # 8 token tiles
NCO = C // 128  # 6 feature tiles
NV = (V + 511) // 512  # 63 vocab chunks (62 full + 1 of 256)

_cache = {}


def _ln_into(nc, pools, h_sb, hnT, idn, i):
    """LayerNorm (stats+normalize, weights pre-folded) of token tile i of h_sb
    into transposed hnT[:, :, i*128:(i+1)*128] (bf16)."""
    small = pools["small"]
    psA = pools["psA"]
    stats = small.tile([128, 3, 6], F32, tag="stats")
    for s in range(3):
        nc.vector.bn_stats(out=stats[:, s, :], in_=h_sb[:, i, s * 256:(s + 1) * 256])
    mv = small.tile([128, 2], F32, tag="mv")
    nc.vector.bn_aggr(out=mv, in_=stats)
    # rstd = exp(-0.5*ln(var+eps)); ln/exp share one ACT table set
    rstd = small.tile([128, 1], F32, tag="rstd")
    nc.scalar.activation(rstd, mv[:, 1:2], AF.Ln, bias=pools["eps"])
    nc.scalar.activation(rstd, rstd, AF.Exp, scale=-0.5)
    nm = small.tile([128, 1], F32, tag="nm")
    nc.vector.tensor_scalar(nm, mv[:, 0:1], rstd, -1.0, ALU.mult, ALU.mult)
    hn_b = small.tile([128, 768], BF16, tag="hn_b")
    nc.scalar.activation(hn_b, h_sb[:, i, :], AF.Identity, bias=nm, scale=rstd)
    pst = psA.tile([128, 768], BF16, tag="pst")
    for co in range(NCO):
        nc.tensor.transpose(pst[:, co * 128:(co + 1) * 128],
                            hn_b[:, co * 128:(co + 1) * 128], idn)
    nc.vector.tensor_copy(
        out=hnT[:, 0:NCO, i * 128:(i + 1) * 128],
        in_=pst.rearrange("p (o t) -> p o t", t=128))


def _build_program(has_qkbias, has_pbias, has_b2bias, has_hbias, debug=False):
    KH = 896 if has_hbias else 768
    KHO = KH // 128
    nc = bacc.Bacc()
    dbg = {}
    if debug:
        dbg["hnT"] = nc.declare_dram_parameter("dbg_hnT", [128, NCO, T], BF16, True)
        dbg["qkh"] = nc.declare_dram_parameter("dbg_qkh", [128, 2, T], BF16, True)
        dbg["vaug"] = nc.declare_dram_parameter("dbg_vaug", [128, NT, H, 98], BF16, True)
        dbg["pt"] = nc.declare_dram_parameter("dbg_pt", [128, T], BF16, True)
        dbg["rsc"] = nc.declare_dram_parameter("dbg_rsc", [128, T], F32, True)
        dbg["rep"] = nc.declare_dram_parameter("dbg_rep", [128, T], BF16, True)
        dbg["attnT"] = nc.declare_dram_parameter("dbg_attnT", [128, H, T], BF16, True)
        dbg["h1"] = nc.declare_dram_parameter("dbg_h1", [128, NT, C], F32, True)

    h0_d = nc.declare_dram_parameter("h0", [T, C], F32, False)
    wqkv_d = nc.declare_dram_parameter("wqkv", [L, C, 2 * CP + C], BF16, False)
    wproj_d = nc.declare_dram_parameter("wproj", [L, CP, C], BF16, False)
    w1_d = nc.declare_dram_parameter("w1", [L, C, F], BF16, False)
    w2_d = nc.declare_dram_parameter("w2", [L, F, C], BF16, False)
    b1_d = nc.declare_dram_parameter("b1", [L, F], F32, False)
    idn_d = nc.declare_dram_parameter("idn", [128, 128], BF16, False)
    ut_d = nc.declare_dram_parameter("ut", [128, 128], BF16, False)
    whead_d = nc.declare_dram_parameter("whead", [KH, V], BF16, False)
    if has_qkbias:
        bqk_d = nc.declare_dram_parameter("bqk", [L, 2 * CP], F32, False)
    if has_pbias:
        pb_d = nc.declare_dram_parameter("pb", [L, C], F32, False)
    if has_b2bias:
        b2_d = nc.declare_dram_parameter("b2", [L, C], F32, False)
    logits_d = nc.declare_dram_parameter("logits", [T, V], F32, True)
    lse_d = nc.declare_dram_parameter("lse", [T], F32, True)

    with tile.TileContext(nc) as tc:
        import contextlib
        stack = contextlib.ExitStack()
        with stack:
            const = stack.enter_context(tc.tile_pool(name="const", bufs=1))
            hnp = stack.enter_context(tc.tile_pool(name="hnp", bufs=1))
            smallp = stack.enter_context(tc.tile_pool(name="small", bufs=4))

            idn = const.tile([128, 128], BF16)
            nc.sync.dma_start(out=idn, in_=idn_d[:, :])
            ut = const.tile([128, 128], BF16)
            nc.sync.dma_start(out=ut, in_=ut_d[:, :])
            h_sb = const.tile([128, NT, C], F32)
            nc.sync.dma_start(out=h_sb, in_=h0_d.rearrange("(i p) c -> p i c", p=128))
            eps_sb = const.tile([128, 1], F32)
            nc.vector.memset(eps_sb, EPS)
            ones96 = const.tile([128, 96], BF16)
            nc.vector.memset(ones96, 1.0)

            # rolling per-layer weight pools: wqkv prefetched one layer ahead
            wq_ctx = {}
            wq_sb = {}

            def open_wqkv(l):
                ctx = tc.tile_pool(name=f"wqkv{l}", bufs=1, side="right")
                pool = ctx.__enter__()
                t = pool.tile([128, NCO, 2 * CP + C], BF16)
                nc.sync.dma_start(
                    out=t, in_=wqkv_d[l].rearrange("(o p) m -> p o m", p=128))
                wq_ctx[l] = ctx
                wq_sb[l] = t

            open_wqkv(0)

            for l in range(L):
                wqkv_sb = wq_sb[l]
                pctx = tc.tile_pool(name=f"wproj{l}", bufs=1, side="right")
                ppool = pctx.__enter__()
                wproj_sb = ppool.tile([128, H, C], BF16)
                nc.sync.dma_start(
                    out=wproj_sb, in_=wproj_d[l].rearrange("(o p) n -> p o n", p=128))
                if has_qkbias:
                    bqk_sb = smallp.tile([128, 16], F32, tag="bqk")
                    nc.sync.dma_start(
                        out=bqk_sb, in_=bqk_d[l].rearrange("(o p) -> p o", p=128))

                # ---- Phase A: LN1 -> hnT ----
                hnT = hnp.tile([128, NCO, T], BF16, tag="hnT")
                with tc.tile_pool(name=f"psA{l}", bufs=2, space="PSUM") as psA:
                    pools = {"small": smallp, "psA": psA, "eps": eps_sb}
                    for i in range(NT):
                        _ln_into(nc, pools, h_sb, hnT, idn, i)

                # ---- Phase B: V (token-major) + V_aug ----
                at_ctx = tc.tile_pool(name=f"atT{l}", bufs=1, side="right")
                atpool = at_ctx.__enter__()
                attnT = atpool.tile([128, H, T], BF16)
                nc.gpsimd.memset(attnT[96:128, :, :], 0.0)
                with tc.tile_pool(name=f"vau{l}", bufs=1) as vpool, \
                     tc.tile_pool(name=f"att{l}", bufs=2) as apool:
                    vaug = vpool.tile([128, NT, H, 98], BF16)
                    nc.gpsimd.memset(vaug[:, :, :, 96:97], 1.0)
                    with tc.tile_pool(name=f"psB{l}", bufs=2, space="PSUM") as psB:
                        for j in range(NT):
                            vps = psB.tile([128, C], F32, tag="vps")
                            for n0, nw in ((0, 512), (512, 256)):
                                for co in range(NCO):
                                    nc.tensor.matmul(
                                        vps[:, n0:n0 + nw],
                                        lhsT=hnT[:, co, j * 128:(j + 1) * 128],
                                        rhs=wqkv_sb[:, co, 2 * CP + n0:2 * CP + n0 + nw],
                                        start=(co == 0), stop=(co == NCO - 1))
                            nc.vector.tensor_copy(
                                out=vaug[:, j, :, 0:96],
                                in_=vps.rearrange("p (h d) -> p h d", d=96))

                    if debug and l == 0:
                        nc.sync.dma_start(out=dbg["hnT"][:, :, :], in_=hnT)
                        nc.sync.dma_start(out=dbg["vaug"][:, :, :, :], in_=vaug)
                    # ---- Phase C: attention per head ----
                    psC_ctx = tc.tile_pool(name=f"psC{l}", bufs=2, space="PSUM")
                    psC = psC_ctx.__enter__()
                    psO_ctx = tc.tile_pool(name=f"psO{l}", bufs=2, space="PSUM")
                    psO = psO_ctx.__enter__()
                    for hh in range(H):
                        qkh = apool.tile([128, 2, T], BF16, tag="qkh")
                        for qk in range(2):
                            ps = psC.tile([128, T], F32, tag="big")
                            for n0 in (0, 512):
                                for co in range(NCO):
                                    nc.tensor.matmul(
                                        ps[:, n0:n0 + 512],
                                        lhsT=wqkv_sb[:, co,
                                                     qk * CP + hh * 128:qk * CP + hh * 128 + 128],
                                        rhs=hnT[:, co, n0:n0 + 512],
                                        start=(co == 0), stop=(co == NCO - 1))
                            if has_qkbias:
                                nc.scalar.activation(
                                    qkh[:, qk, :], ps, AF.Identity,
                                    bias=bqk_sb[:, qk * 8 + hh:qk * 8 + hh + 1])
                            else:
                                nc.vector.tensor_copy(out=qkh[:, qk, :], in_=ps)

                        ops = psO.tile([128, T], F32, tag="ops")
                        for j in range(NT):
                            q0 = j * 128
                            st = psC.tile([128, T], F32, tag="big")
                            for c0 in range(q0 // 512 * 512, T, 512):
                                a = max(q0, c0)
                                nc.tensor.matmul(
                                    st[:, a:c0 + 512],
                                    lhsT=qkh[:, 1, q0:q0 + 128],
                                    rhs=qkh[:, 0, a:c0 + 512],
                                    start=True, stop=True)
                            pt = apool.tile([128, T], BF16, tag="pt")
                            nc.scalar.activation(pt[:, q0:T], st[:, q0:T], AF.Exp,
                                                 scale=float(D) ** -0.5)
                            nc.vector.tensor_tensor(
                                out=pt[:, q0:q0 + 128], in0=pt[:, q0:q0 + 128],
                                in1=ut, op=ALU.mult)
                            if debug and l == 0 and hh == 0 and j == 0:
                                nc.sync.dma_start(out=dbg["pt"][:, :], in_=pt)
                            for c0 in range(0, T, 512):
                                if c0 + 512 <= q0:
                                    continue
                                a = max(q0, c0)
                                nc.tensor.matmul(
                                    ops[0:97, a:c0 + 512],
                                    lhsT=vaug[:, j, hh, 0:97],
                                    rhs=pt[:, a:c0 + 512],
                                    start=(j == 0),
                                    stop=(j == min(7, (c0 + 511) // 128)))
                        # softmax denominators: row 96 of ops
                        rs = apool.tile([128, T], F32, tag="rs")
                        nc.scalar.activation(rs[96:97, :], ops[96:97, :], AF.Ln)
                        rsb = apool.tile([128, T], BF16, tag="rsb")
                        nc.scalar.activation(rsb[96:97, :], rs[96:97, :], AF.Exp,
                                             scale=-1.0)
                        repp = psC.tile([128, T], F32, tag="big")
                        for c0 in (0, 512):
                            nc.tensor.matmul(
                                repp[0:96, c0:c0 + 512],
                                lhsT=ones96[96:97, :],
                                rhs=rsb[96:97, c0:c0 + 512],
                                start=True, stop=True, tile_position=(96, 0))
                        rep = apool.tile([128, T], BF16, tag="rep")
                        nc.vector.tensor_copy(out=rep[0:96, :], in_=repp[0:96, :])
                        nc.vector.tensor_tensor(
                            out=attnT[0:96, hh, :], in0=ops[0:96, :],
                            in1=rep[0:96, :], op=ALU.mult)
                        if debug and l == 0 and hh == 0:
                            nc.sync.dma_start(out=dbg["qkh"][:, :, :], in_=qkh)
                            osc = apool.tile([128, T], F32, tag="osc")
                            nc.vector.tensor_copy(out=osc, in_=ops)
                            nc.sync.dma_start(out=dbg["rsc"][:, :], in_=osc)
                            nc.sync.dma_start(out=dbg["rep"][:, :], in_=rep)
                    psO_ctx.__exit__(None, None, None)
                    psC_ctx.__exit__(None, None, None)
                    if debug and l == 0:
                        nc.sync.dma_start(out=dbg["attnT"][:, :, :], in_=attnT)

                # ---- Phase D: proj + residual ----
                with tc.tile_pool(name=f"psD{l}", bufs=2, space="PSUM") as psD:
                    if has_pbias:
                        pbrow = smallp.tile([128, C], F32, tag="pbrow")
                        nc.sync.dma_start(out=pbrow[0:1, :], in_=pb_d[l][None, :])
                        pbrep = smallp.tile([128, C], F32, tag="pbrep")
                        nc.gpsimd.partition_broadcast(pbrep, pbrow[0:1, :])
                    for i in range(NT):
                        pp = psD.tile([128, C], F32, tag="pp")
                        for n0, nw in ((0, 512), (512, 256)):
                            for kt in range(H):
                                nc.tensor.matmul(
                                    pp[:, n0:n0 + nw],
                                    lhsT=attnT[:, kt, i * 128:(i + 1) * 128],
                                    rhs=wproj_sb[:, kt, n0:n0 + nw],
                                    start=(kt == 0), stop=(kt == H - 1))
                        nc.vector.tensor_add(out=h_sb[:, i, :], in0=h_sb[:, i, :],
                                             in1=pp)
                        if has_pbias:
                            nc.vector.tensor_add(out=h_sb[:, i, :],
                                                 in0=h_sb[:, i, :], in1=pbrep)
                if debug and l == 0:
                    nc.sync.dma_start(out=dbg["h1"][:, :, :], in_=h_sb)
                at_ctx.__exit__(None, None, None)
                pctx.__exit__(None, None, None)
                wq_ctx[l].__exit__(None, None, None)

                # ---- Phase E: LN2 -> hnT ----
                hnT = hnp.tile([128, NCO, T], BF16, tag="hnT")
                with tc.tile_pool(name=f"psE{l}", bufs=2, space="PSUM") as psA:
                    pools = {"small": smallp, "psA": psA, "eps": eps_sb}
                    for i in range(NT):
                        _ln_into(nc, pools, h_sb, hnT, idn, i)

                # ---- Phase F: MLP ----
                wctx = tc.tile_pool(name=f"wffn{l}", bufs=1)
                wpool = wctx.__enter__()
                w1_sb = wpool.tile([128, NCO, F], BF16, tag="w1")
                nc.sync.dma_start(
                    out=w1_sb, in_=w1_d[l].rearrange("(o p) m -> p o m", p=128))
                w2_sb = wpool.tile([128, F // 128, C], BF16, tag="w2")
                nc.sync.dma_start(
                    out=w2_sb, in_=w2_d[l].rearrange("(o p) n -> p o n", p=128))
                b1_sb = smallp.tile([128, F // 128], F32, tag="b1")
                nc.sync.dma_start(
                    out=b1_sb, in_=b1_d[l].rearrange("(o p) -> p o", p=128))
                if has_b2bias:
                    b2row = smallp.tile([128, C], F32, tag="b2row")
                    nc.sync.dma_start(out=b2row[0:1, :], in_=b2_d[l][None, :])
                    b2rep = smallp.tile([128, C], F32, tag="b2rep")
                    nc.gpsimd.partition_broadcast(b2rep, b2row[0:1, :])

                with tc.tile_pool(name=f"fc{l}", bufs=1) as fcp, \
                     tc.tile_pool(name=f"psF1{l}", bufs=3, space="PSUM") as psF1, \
                     tc.tile_pool(name=f"psF2{l}", bufs=2, space="PSUM") as psF2:
                    for quarter in range(4):
                        t0 = quarter * 256
                        fc1 = fcp.tile([128, F // 128, 256], BF16, tag="fc1")
                        for m in range(F // 128):
                            fp = psF1.tile([128, 256], F32, tag="fp")
                            for co in range(NCO):
                                nc.tensor.matmul(
                                    fp, lhsT=w1_sb[:, co, m * 128:(m + 1) * 128],
                                    rhs=hnT[:, co, t0:t0 + 256],
                                    start=(co == 0), stop=(co == NCO - 1))
                            nc.scalar.activation(fc1[:, m, :], fp, AF.Relu,
                                                 bias=b1_sb[:, m:m + 1])
                        # prefetch next layer's qkv weights
                        if quarter == 0 and l + 1 < L:
                            open_wqkv(l + 1)
                        for ii in range(2):
                            i = quarter * 2 + ii
                            p2 = psF2.tile([128, C], F32, tag="p2")
                            for n0, nw in ((0, 512), (512, 256)):
                                for kt in range(F // 128):
                                    nc.tensor.matmul(
                                        p2[:, n0:n0 + nw],
                                        lhsT=fc1[:, kt, ii * 128:(ii + 1) * 128],
                                        rhs=w2_sb[:, kt, n0:n0 + nw],
                                        start=(kt == 0), stop=(kt == F // 128 - 1))
                            nc.vector.tensor_add(out=h_sb[:, i, :],
                                                 in0=h_sb[:, i, :], in1=p2)
                            if has_b2bias:
                                nc.vector.tensor_add(out=h_sb[:, i, :],
                                                     in0=h_sb[:, i, :], in1=b2rep)
                wctx.__exit__(None, None, None)

            # ---- Phase G: final LN + lm_head + logsumexp ----
            hfp = stack.enter_context(tc.tile_pool(name="hfp", bufs=1))
            hfT = hfp.tile([128, KHO, T], BF16, tag="hnT2")
            with tc.tile_pool(name="psLNf", bufs=2, space="PSUM") as psA:
                pools = {"small": smallp, "psA": psA, "eps": eps_sb}
                for i in range(NT):
                    _ln_into(nc, pools, h_sb, hfT, idn, i)
            if has_hbias:
                nc.vector.memset(hfT[:, 6, :], 0.0)
                nc.vector.memset(hfT[0:1, 6, :], 1.0)

            acc = hfp.tile([128, NT, NV], F32)
            with tc.tile_pool(name="whp", bufs=3) as whp, \
                 tc.tile_pool(name="lgp", bufs=6) as lgp, \
                 tc.tile_pool(name="exg", bufs=2) as exg, \
                 tc.tile_pool(name="psG", bufs=6, space="PSUM") as psG:
                for n in range(NV):
                    n0 = n * 512
                    nw = min(512, V - n0)
                    wh = whp.tile([128, KHO, 512], BF16, tag="wh")
                    nc.sync.dma_start(
                        out=wh[:, :, 0:nw],
                        in_=whead_d[:, n0:n0 + nw].rearrange("(o p) m -> p o m", p=128))
                    for i in range(NT):
                        lp = psG.tile([128, 512], F32, tag="lp")
                        for co in range(KHO):
                            nc.tensor.matmul(
                                lp[:, 0:nw], lhsT=hfT[:, co, i * 128:(i + 1) * 128],
                                rhs=wh[:, co, 0:nw],
                                start=(co == 0), stop=(co == KHO - 1))
                        lg = lgp.tile([128, 512], F32, tag="lg")
                        nc.vector.tensor_copy(out=lg[:, 0:nw], in_=lp[:, 0:nw])
                        nc.sync.dma_start(
                            out=logits_d[i * 128:(i + 1) * 128, n0:n0 + nw],
                            in_=lg[:, 0:nw])
                        ex = exg.tile([128, 512], BF16, tag="ex")
                        nc.scalar.activation(ex[:, 0:nw], lg[:, 0:nw], AF.Exp,
                                             accum_out=acc[:, i, n:n + 1])
                lse_sb = hfp.tile([128, NT], F32)
                for i in range(NT):
                    tot = smallp.tile([128, 1], F32, tag="tot")
                    nc.vector.tensor_reduce(tot, acc[:, i, :], AX.X, ALU.add)
                    nc.scalar.activation(lse_sb[:, i:i + 1], tot, AF.Ln)
                nc.sync.dma_start(out=lse_d.rearrange("(i p) -> p i", p=128),
                                  in_=lse_sb)

    nc.finalize()
    return nc


def kernel(x, y, tok_emb, pos_emb, ln1_w, ln1_b, attn_w, attn_b, proj_w, proj_b,
           ln2_w, ln2_b, ffn_w1, ffn_b1, ffn_w2, ffn_b2, lnf_w, lnf_b,
           head_w, head_b):
    x = np.asarray(x)
    y = np.asarray(y)
    f = lambda a: np.asarray(a, np.float32)
    tok_emb, pos_emb = f(tok_emb), f(pos_emb)
    ln1_w, ln1_b, attn_w, attn_b = f(ln1_w), f(ln1_b), f(attn_w), f(attn_b)
    proj_w, proj_b, ln2_w, ln2_b = f(proj_w), f(proj_b), f(ln2_w), f(ln2_b)
    ffn_w1, ffn_b1, ffn_w2, ffn_b2 = f(ffn_w1), f(ffn_b1), f(ffn_w2), f(ffn_b2)
    lnf_w, lnf_b, head_w, head_b = f(lnf_w), f(lnf_b), f(head_w), f(head_b)

    bf = lambda a: np.ascontiguousarray(a.astype(ml_dtypes.bfloat16))

    # ---- host preprocessing: embeddings + LN folding + head-dim padding ----
    h0 = tok_emb[x] + pos_emb[None, :, :]                      # [B,T,C] f32

    WQKV = np.zeros((L, C, 2 * CP + C), np.float32)
    BQK = np.zeros((L, 2 * CP), np.float32)
    WPROJ = np.zeros((L, CP, C), np.float32)
    PB = np.zeros((L, C), np.float32)
    W1 = np.zeros((L, C, F), np.float32)
    B1 = np.zeros((L, F), np.float32)
    for l in range(L):
        aw = ln1_w[l][:, None] * attn_w[l]                     # [C, 3C]
        ab = attn_b[l] + ln1_b[l] @ attn_w[l]                  # [3C]
        for hh in range(H):
            WQKV[l, :, hh * 128:hh * 128 + D] = aw[:, hh * D:(hh + 1) * D]
            WQKV[l, :, CP + hh * 128:CP + hh * 128 + D] = aw[:, C + hh * D:C + (hh + 1) * D]
            BQK[l, hh * 128:hh * 128 + D] = ab[hh * D:(hh + 1) * D]
            BQK[l, CP + hh * 128:CP + hh * 128 + D] = ab[C + hh * D:C + (hh + 1) * D]
            WPROJ[l, hh * 128:hh * 128 + D, :] = proj_w[l][hh * D:(hh + 1) * D, :]
        WQKV[l, :, 2 * CP:] = aw[:, 2 * C:]                    # v (unpadded)
        PB[l] = proj_b[l] + ab[2 * C:] @ proj_w[l]             # v-bias folded
        W1[l] = ln2_w[l][:, None] * ffn_w1[l]
        B1[l] = ffn_b1[l] + ln2_b[l] @ ffn_w1[l]
    WHE = lnf_w[:, None] * head_w                              # [C, V]
    HB = head_b + lnf_b @ head_w                               # [V]

    has_qkbias = bool(np.any(BQK))
    has_pbias = bool(np.any(PB))
    has_b2bias = bool(np.any(ffn_b2))
    has_hbias = bool(np.any(HB))
    if has_hbias:
        WHEAD = np.zeros((896, V), np.float32)
        WHEAD[:C] = WHE
        WHEAD[C] = HB
    else:
        WHEAD = WHE

    debug = os.environ.get("KERNEL_DEBUG") == "1"
    key = (has_qkbias, has_pbias, has_b2bias, has_hbias, debug)
    if key not in _cache:
        _cache[key] = _build_program(*key[:4], debug=debug)
    nc = _cache[key]

    shared = {
        "wqkv": bf(WQKV), "wproj": bf(WPROJ), "w1": bf(W1), "w2": bf(ffn_w2),
        "b1": np.ascontiguousarray(B1), "whead": bf(WHEAD),
        "idn": np.eye(128, dtype=ml_dtypes.bfloat16),
        "ut": np.triu(np.ones((128, 128), ml_dtypes.bfloat16)),
    }
    if has_qkbias:
        shared["bqk"] = np.ascontiguousarray(BQK)
    if has_pbias:
        shared["pb"] = np.ascontiguousarray(PB)
    if has_b2bias:
        shared["b2"] = np.ascontiguousarray(ffn_b2)

    in_maps = [dict(shared, h0=np.ascontiguousarray(h0[c])) for c in range(B)]

    trace = os.environ.get("KERNEL_TRACE") == "1"
    res = run_bass_kernel_spmd(nc, in_maps, core_ids=list(range(NCORES)),
                               trace=trace)
    if trace and res.exec_time_ns is not None:
        print(f"HW exec time: {res.exec_time_ns} ns")
        kernel.last_exec_time_ns = res.exec_time_ns

    if debug:
        kernel.debug_out = res.results[0]
    logits = np.stack([res.results[c]["logits"] for c in range(B)])   # [B,T,V]
    lse = np.stack([res.results[c]["lse"] for c in range(B)])          # [B,T]
    ly = np.take_along_axis(logits.reshape(B * T, V),
                            y.reshape(B * T, 1).astype(np.int64), axis=1)[:, 0]
    loss = np.float32(np.mean(lse.reshape(B * T) - ly))
    return logits, loss


# revision 13
# speedup vs baseline: 1.1491x; 1.1491x over previous
"""Bass/Trainium2 kernel for nn_BigramLanguageModel (6-layer GPT, B=8,T=1024,C=768,V=32000).

Strategy: data-parallel over batch across the 8 NeuronCores (one batch element
per core, no collectives). Each core runs the full transformer + lm_head +
logsumexp for its sequence; the host folds LN weights/biases into adjacent
matmul weights, gathers embeddings, and combines per-core loss partials.
"""

import os
import numpy as np
import ml_dtypes

import concourse.bass as bass
import concourse.mybir as mybir
import concourse.tile as tile
from concourse import bacc
from concourse.bass_utils import run_bass_kernel_spmd

_orig_get_act_tables = bacc.get_activation_tables

def _steered_act_tables(arch):
    tabs = {k: set(v) for k, v in _orig_get_act_tables(arch).items()}
    combined = tabs.get("natural_log_exp_and_others")
    if combined and AF.Exp in combined and AF.Ln in combined:
        for name, fns in tabs.items():
            if name != "natural_log_exp_and_others":
                fns.discard(AF.Exp)
                fns.discard(AF.Ln)
    return tabs

bacc.get_activation_tables = _steered_act_tables

F32 = mybir.dt.float32
BF16 = mybir.dt.bfloat16
AF = mybir.ActivationFunctionType
ALU = mybir.AluOpType
AX = mybir.AxisListType

V = 32000
T = 1024
C = 768
L = 6
H = 8
B = 8
D = 96          # real head dim
DP = 128        # padded head dim
CP = H * DP     # 1024, padded attn-concat dim
F = 4 * C       # 3072
EPS = 1e-5
NCORES = 8
NT = T // 128   # 8 token tiles
NCO = C // 128  # 6 feature tiles
NV = (V + 511) // 512  # 63 vocab chunks (62 full + 1 of 256)

_cache = {}


def _ln_into(nc, pools, h_sb, hnT, idn, i):
    """LayerNorm (stats+normalize, weights pre-folded) of token tile i of h_sb
    into transposed hnT[:, :, i*128:(i+1)*128] (bf16)."""
    small = pools["small"]
    psA = pools["psA"]
    stats = small.tile([128, 3, 6], F32, tag="stats")
    for s in range(3):
        nc.vector.bn_stats(out=stats[:, s, :], in_=h_sb[:, i, s * 256:(s + 1) * 256])
    mv = small.tile([128, 2], F32, tag="mv")
    nc.vector.bn_aggr(out=mv, in_=stats)
    # rstd = exp(-0.5*ln(var+eps)); ln/exp share one ACT table set
    rstd = small.tile([128, 1], F32, tag="rstd")
    nc.scalar.activation(rstd, mv[:, 1:2], AF.Ln, bias=pools["eps"])
    nc.scalar.activation(rstd, rstd, AF.Exp, scale=-0.5)
    nm = small.tile([128, 1], F32, tag="nm")
    nc.vector.tensor_scalar(nm, mv[:, 0:1], rstd, -1.0, ALU.mult, ALU.mult)
    hn_b = small.tile([128, 768], BF16, tag="hn_b")
    nc.scalar.activation(hn_b, h_sb[:, i, :], AF.Identity, bias=nm, scale=rstd)
    pst = psA.tile([128, 768], BF16, tag="pst")
    for co in range(NCO):
        nc.tensor.transpose(pst[:, co * 128:(co + 1) * 128],
                            hn_b[:, co * 128:(co + 1) * 128], idn)
    nc.vector.tensor_copy(
        out=hnT[:, 0:NCO, i * 128:(i + 1) * 128],
        in_=pst.rearrange("p (o t) -> p o t", t=128))


def _build_program(has_qkbias, has_pbias, has_b2bias, has_hbias, debug=False):
    KH = 896 if has_hbias else 768
    KHO = KH // 128
    nc = bacc.Bacc()
    dbg = {}
    if debug:
        dbg["hnT"] = nc.declare_dram_parameter("dbg_hnT", [128, NCO, T], BF16, True)
        dbg["qkh"] = nc.declare_dram_parameter("dbg_qkh", [128, 2, T], BF16, True)
        dbg["vaug"] = nc.declare_dram_parameter("dbg_vaug", [128, NT, H, 98], BF16, True)
        dbg["pt"] = nc.declare_dram_parameter("dbg_pt", [128, T], BF16, True)
        dbg["rsc"] = nc.declare_dram_parameter("dbg_rsc", [128, T], F32, True)
        dbg["rep"] = nc.declare_dram_parameter("dbg_rep", [128, T], BF16, True)
        dbg["attnT"] = nc.declare_dram_parameter("dbg_attnT", [128, H, T], BF16, True)
        dbg["h1"] = nc.declare_dram_parameter("dbg_h1", [128, NT, C], F32, True)

    h0_d = nc.declare_dram_parameter("h0", [T, C], F32, False)
    wqkv_d = nc.declare_dram_parameter("wqkv", [L, C, 2 * CP + C], BF16, False)
    wproj_d = nc.declare_dram_parameter("wproj", [L, CP, C], BF16, False)
    w1_d = nc.declare_dram_parameter("w1", [L, C, F], BF16, False)
    w2_d = nc.declare_dram_parameter("w2", [L, F, C], BF16, False)
    b1_d = nc.declare_dram_parameter("b1", [L, F], F32, False)
    idn_d = nc.declare_dram_parameter("idn", [128, 128], BF16, False)
    ut_d = nc.declare_dram_parameter("ut", [128, 128], BF16, False)
    whead_d = nc.declare_dram_parameter("whead", [KH, V], BF16, False)
    if has_qkbias:
        bqk_d = nc.declare_dram_parameter("bqk", [L, 2 * CP], F32, False)
    if has_pbias:
        pb_d = nc.declare_dram_parameter("pb", [L, C], F32, False)
    if has_b2bias:
        b2_d = nc.declare_dram_parameter("b2", [L, C], F32, False)
    logits_d = nc.declare_dram_parameter("logits", [T, V], F32, True)
    lse_d = nc.declare_dram_parameter("lse", [T], F32, True)

    with tile.TileContext(nc) as tc:
        import contextlib
        stack = contextlib.ExitStack()
        with stack:
            const = stack.enter_context(tc.tile_pool(name="const", bufs=1))
            hnp = stack.enter_context(tc.tile_pool(name="hnp", bufs=1))
            smallp = stack.enter_context(tc.tile_pool(name="small", bufs=4))

            idn = const.tile([128, 128], BF16)
            nc.sync.dma_start(out=idn, in_=idn_d[:, :])
            ut = const.tile([128, 128], BF16)
            nc.sync.dma_start(out=ut, in_=ut_d[:, :])
            h_sb = const.tile([128, NT, C], F32)
            nc.sync.dma_start(out=h_sb, in_=h0_d.rearrange("(i p) c -> p i c", p=128))
            eps_sb = const.tile([128, 1], F32)
            nc.vector.memset(eps_sb, EPS)
            ones96 = const.tile([128, 96], BF16)
            nc.vector.memset(ones96, 1.0)

            # rolling per-layer weight pools: wqkv prefetched one layer ahead
            wq_ctx = {}
            wq_sb = {}

            def open_wqkv(l):
                ctx = tc.tile_pool(name=f"wqkv{l}", bufs=1, side="right")
                pool = ctx.__enter__()
                t = pool.tile([128, NCO, 2 * CP + C], BF16)
                nc.sync.dma_start(
                    out=t, in_=wqkv_d[l].rearrange("(o p) m -> p o m", p=128))
                wq_ctx[l] = ctx
                wq_sb[l] = t

            open_wqkv(0)

            for l in range(L):
                wqkv_sb = wq_sb[l]
                pctx = tc.tile_pool(name=f"wproj{l}", bufs=1, side="right")
                ppool = pctx.__enter__()
                wproj_sb = ppool.tile([128, H, C], BF16)
                nc.sync.dma_start(
                    out=wproj_sb, in_=wproj_d[l].rearrange("(o p) n -> p o n", p=128))
                if has_qkbias:
                    bqk_sb = smallp.tile([128, 16], F32, tag="bqk")
                    nc.sync.dma_start(
                        out=bqk_sb, in_=bqk_d[l].rearrange("(o p) -> p o", p=128))

                # ---- Phase A: LN1 -> hnT ----
                hnT = hnp.tile([128, NCO, T], BF16, tag="hnT")
                with tc.tile_pool(name=f"psA{l}", bufs=2, space="PSUM") as psA:
                    pools = {"small": smallp, "psA": psA, "eps": eps_sb}
                    for i in range(NT):
                        _ln_into(nc, pools, h_sb, hnT, idn, i)

                # ---- Phase B: V (token-major) + V_aug ----
                at_ctx = tc.tile_pool(name=f"atT{l}", bufs=1, side="right")
                atpool = at_ctx.__enter__()
                attnT = atpool.tile([128, H, T], BF16)
                nc.gpsimd.memset(attnT[96:128, :, :], 0.0)
                with tc.tile_pool(name=f"vau{l}", bufs=1) as vpool, \
                     tc.tile_pool(name=f"att{l}", bufs=3) as apool:
                    vaug = vpool.tile([128, NT, H, 98], BF16)
                    nc.gpsimd.memset(vaug[:, :, :, 96:97], 1.0)
                    with tc.tile_pool(name=f"psB{l}", bufs=2, space="PSUM") as psB:
                        for j in range(NT):
                            vps = psB.tile([128, C], F32, tag="vps")
                            for n0, nw in ((0, 512), (512, 256)):
                                for co in range(NCO):
                                    nc.tensor.matmul(
                                        vps[:, n0:n0 + nw],
                                        lhsT=hnT[:, co, j * 128:(j + 1) * 128],
                                        rhs=wqkv_sb[:, co, 2 * CP + n0:2 * CP + n0 + nw],
                                        start=(co == 0), stop=(co == NCO - 1))
                            nc.vector.tensor_copy(
                                out=vaug[:, j, :, 0:96],
                                in_=vps.rearrange("p (h d) -> p h d", d=96))

                    if debug and l == 0:
                        nc.sync.dma_start(out=dbg["hnT"][:, :, :], in_=hnT)
                        nc.sync.dma_start(out=dbg["vaug"][:, :, :, :], in_=vaug)
                    # ---- Phase C: attention per head ----
                    psC_ctx = tc.tile_pool(name=f"psC{l}", bufs=4, space="PSUM")
                    psC = psC_ctx.__enter__()
                    psO_ctx = tc.tile_pool(name=f"psO{l}", bufs=2, space="PSUM")
                    psO = psO_ctx.__enter__()
                    for hh in range(H):
                        qkh = apool.tile([128, 2, T], BF16, tag="qkh")
                        for qk in range(2):
                            for n0 in (0, 512):
                                ps = psC.tile([128, 512], F32, tag="big")
                                for co in range(NCO):
                                    nc.tensor.matmul(
                                        ps,
                                        lhsT=wqkv_sb[:, co,
                                                     qk * CP + hh * 128:qk * CP + hh * 128 + 128],
                                        rhs=hnT[:, co, n0:n0 + 512],
                                        start=(co == 0), stop=(co == NCO - 1))
                                if has_qkbias:
                                    nc.scalar.activation(
                                        qkh[:, qk, n0:n0 + 512], ps, AF.Identity,
                                        bias=bqk_sb[:, qk * 8 + hh:qk * 8 + hh + 1])
                                else:
                                    nc.vector.tensor_copy(
                                        out=qkh[:, qk, n0:n0 + 512], in_=ps)

                        ops = psO.tile([128, T], F32, tag="ops")
                        for j in range(NT):
                            q0 = j * 128
                            pt = apool.tile([128, T], BF16, tag="pt")
                            for c0 in range(q0 // 512 * 512, T, 512):
                                a = max(q0, c0)
                                st = psC.tile([128, 512], F32, tag="big")
                                nc.tensor.matmul(
                                    st[:, 0:c0 + 512 - a],
                                    lhsT=qkh[:, 1, q0:q0 + 128],
                                    rhs=qkh[:, 0, a:c0 + 512],
                                    start=True, stop=True)
                                nc.scalar.activation(pt[:, a:c0 + 512],
                                                     st[:, 0:c0 + 512 - a], AF.Exp,
                                                     scale=float(D) ** -0.5)
                            nc.vector.tensor_tensor(
                                out=pt[:, q0:q0 + 128], in0=pt[:, q0:q0 + 128],
                                in1=ut, op=ALU.mult)
                            if debug and l == 0 and hh == 0 and j == 0:
                                nc.sync.dma_start(out=dbg["pt"][:, :], in_=pt)
                            for c0 in range(0, T, 512):
                                if c0 + 512 <= q0:
                                    continue
                                a = max(q0, c0)
                                nc.tensor.matmul(
                                    ops[0:97, a:c0 + 512],
                                    lhsT=vaug[:, j, hh, 0:97],
                                    rhs=pt[:, a:c0 + 512],
                                    start=(j == 0),
                                    stop=(j == min(7, (c0 + 511) // 128)))
                        # softmax denominators: row 96 of ops
                        rs = apool.tile([128, T], F32, tag="rs")
                        nc.scalar.activation(rs[96:97, :], ops[96:97, :], AF.Ln)
                        rsb = apool.tile([128, T], BF16, tag="rsb")
                        nc.scalar.activation(rsb[96:97, :], rs[96:97, :], AF.Exp,
                                             scale=-1.0)
                        rep = apool.tile([128, T], BF16, tag="rep")
                        for c0 in (0, 512):
                            repp = psC.tile([128, 512], F32, tag="big")
                            nc.tensor.matmul(
                                repp[0:96, :],
                                lhsT=ones96[96:97, :],
                                rhs=rsb[96:97, c0:c0 + 512],
                                start=True, stop=True, tile_position=(96, 0))
                            nc.vector.tensor_copy(out=rep[0:96, c0:c0 + 512],
                                                  in_=repp[0:96, :])
                        nc.vector.tensor_tensor(
                            out=attnT[0:96, hh, :], in0=ops[0:96, :],
                            in1=rep[0:96, :], op=ALU.mult)
                        if debug and l == 0 and hh == 0:
                            nc.sync.dma_start(out=dbg["qkh"][:, :, :], in_=qkh)
                            osc = apool.tile([128, T], F32, tag="osc")
                            nc.vector.tensor_copy(out=osc, in_=ops)
                            nc.sync.dma_start(out=dbg["rsc"][:, :], in_=osc)
                            nc.sync.dma_start(out=dbg["rep"][:, :], in_=rep)
                    psO_ctx.__exit__(None, None, None)
                    psC_ctx.__exit__(None, None, None)
                    if debug and l == 0:
                        nc.sync.dma_start(out=dbg["attnT"][:, :, :], in_=attnT)

                # ---- Phase D: proj + residual ----
                with tc.tile_pool(name=f"psD{l}", bufs=2, space="PSUM") as psD:
                    if has_pbias:
                        pbrow = smallp.tile([128, C], F32, tag="pbrow")
                        nc.sync.dma_start(out=pbrow[0:1, :], in_=pb_d[l][None, :])
                        pbrep = smallp.tile([128, C], F32, tag="pbrep")
                        nc.gpsimd.partition_broadcast(pbrep, pbrow[0:1, :])
                    for i in range(NT):
                        pp = psD.tile([128, C], F32, tag="pp")
                        for n0, nw in ((0, 512), (512, 256)):
                            for kt in range(H):
                                nc.tensor.matmul(
                                    pp[:, n0:n0 + nw],
                                    lhsT=attnT[:, kt, i * 128:(i + 1) * 128],
                                    rhs=wproj_sb[:, kt, n0:n0 + nw],
                                    start=(kt == 0), stop=(kt == H - 1))
                        nc.vector.tensor_add(out=h_sb[:, i, :], in0=h_sb[:, i, :],
                                             in1=pp)
                        if has_pbias:
                            nc.vector.tensor_add(out=h_sb[:, i, :],
                                                 in0=h_sb[:, i, :], in1=pbrep)
                if debug and l == 0:
                    nc.sync.dma_start(out=dbg["h1"][:, :, :], in_=h_sb)
                at_ctx.__exit__(None, None, None)
                pctx.__exit__(None, None, None)
                wq_ctx[l].__exit__(None, None, None)

                # ---- Phase E: LN2 -> hnT ----
                hnT = hnp.tile([128, NCO, T], BF16, tag="hnT")
                with tc.tile_pool(name=f"psE{l}", bufs=2, space="PSUM") as psA:
                    pools = {"small": smallp, "psA": psA, "eps": eps_sb}
                    for i in range(NT):
                        _ln_into(nc, pools, h_sb, hnT, idn, i)

                # ---- Phase F: MLP ----
                wctx = tc.tile_pool(name=f"wffn{l}", bufs=1)
                wpool = wctx.__enter__()
                w1_sb = wpool.tile([128, NCO, F], BF16, tag="w1")
                nc.sync.dma_start(
                    out=w1_sb, in_=w1_d[l].rearrange("(o p) m -> p o m", p=128))
                w2_sb = wpool.tile([128, F // 128, C], BF16, tag="w2")
                nc.sync.dma_start(
                    out=w2_sb, in_=w2_d[l].rearrange("(o p) n -> p o n", p=128))
                b1_sb = smallp.tile([128, F // 128], F32, tag="b1")
                nc.sync.dma_start(
                    out=b1_sb, in_=b1_d[l].rearrange("(o p) -> p o", p=128))
                if has_b2bias:
                    b2row = smallp.tile([128, C], F32, tag="b2row")
                    nc.sync.dma_start(out=b2row[0:1, :], in_=b2_d[l][None, :])
                    b2rep = smallp.tile([128, C], F32, tag="b2rep")
                    nc.gpsimd.partition_broadcast(b2rep, b2row[0:1, :])

                with tc.tile_pool(name=f"fc{l}", bufs=1) as fcp, \
                     tc.tile_pool(name=f"psF1{l}", bufs=3, space="PSUM") as psF1, \
                     tc.tile_pool(name=f"psF2{l}", bufs=2, space="PSUM") as psF2:
                    for quarter in range(4):
                        t0 = quarter * 256
                        fc1 = fcp.tile([128, F // 128, 256], BF16, tag="fc1")
                        for m in range(F // 128):
                            fp = psF1.tile([128, 256], F32, tag="fp")
                            for co in range(NCO):
                                nc.tensor.matmul(
                                    fp, lhsT=w1_sb[:, co, m * 128:(m + 1) * 128],
                                    rhs=hnT[:, co, t0:t0 + 256],
                                    start=(co == 0), stop=(co == NCO - 1))
                            nc.scalar.activation(fc1[:, m, :], fp, AF.Relu,
                                                 bias=b1_sb[:, m:m + 1])
                        # prefetch next layer's qkv weights
                        if quarter == 0 and l + 1 < L:
                            open_wqkv(l + 1)
                        for ii in range(2):
                            i = quarter * 2 + ii
                            p2 = psF2.tile([128, C], F32, tag="p2")
                            for n0, nw in ((0, 512), (512, 256)):
                                for kt in range(F // 128):
                                    nc.tensor.matmul(
                                        p2[:, n0:n0 + nw],
                                        lhsT=fc1[:, kt, ii * 128:(ii + 1) * 128],
                                        rhs=w2_sb[:, kt, n0:n0 + nw],
                                        start=(kt == 0), stop=(kt == F // 128 - 1))
                            nc.vector.tensor_add(out=h_sb[:, i, :],
                                                 in0=h_sb[:, i, :], in1=p2)
                            if has_b2bias:
                                nc.vector.tensor_add(out=h_sb[:, i, :],
                                                     in0=h_sb[:, i, :], in1=b2rep)
                wctx.__exit__(None, None, None)

            # ---- Phase G: final LN + lm_head + logsumexp ----
            hfp = stack.enter_context(tc.tile_pool(name="hfp", bufs=1))
            hfT = hfp.tile([128, KHO, T], BF16, tag="hnT2")
            with tc.tile_pool(name="psLNf", bufs=2, space="PSUM") as psA:
                pools = {"small": smallp, "psA": psA, "eps": eps_sb}
                for i in range(NT):
                    _ln_into(nc, pools, h_sb, hfT, idn, i)
            if has_hbias:
                nc.vector.memset(hfT[:, 6, :], 0.0)
                nc.vector.memset(hfT[0:1, 6, :], 1.0)

            acc = hfp.tile([128, NT, NV], F32)
            with tc.tile_pool(name="whp", bufs=3) as whp, \
                 tc.tile_pool(name="lgp", bufs=6) as lgp, \
                 tc.tile_pool(name="exg", bufs=2) as exg, \
                 tc.tile_pool(name="psG", bufs=6, space="PSUM") as psG:
                for n in range(NV):
                    n0 = n * 512
                    nw = min(512, V - n0)
                    wh = whp.tile([128, KHO, 512], BF16, tag="wh")
                    nc.sync.dma_start(
                        out=wh[:, :, 0:nw],
                        in_=whead_d[:, n0:n0 + nw].rearrange("(o p) m -> p o m", p=128))
                    for i in range(NT):
                        lp = psG.tile([128, 512], F32, tag="lp")
                        for co in range(KHO):
                            nc.tensor.matmul(
                                lp[:, 0:nw], lhsT=hfT[:, co, i * 128:(i + 1) * 128],
                                rhs=wh[:, co, 0:nw],
                                start=(co == 0), stop=(co == KHO - 1))
                        lg = lgp.tile([128, 512], F32, tag="lg")
                        nc.vector.tensor_copy(out=lg[:, 0:nw], in_=lp[:, 0:nw])
                        nc.sync.dma_start(
                            out=logits_d[i * 128:(i + 1) * 128, n0:n0 + nw],
                            in_=lg[:, 0:nw])
                        ex = exg.tile([128, 512], BF16, tag="ex")
                        nc.scalar.activation(ex[:, 0:nw], lg[:, 0:nw], AF.Exp,
                                             accum_out=acc[:, i, n:n + 1])
                lse_sb = hfp.tile([128, NT], F32)
                for i in range(NT):
                    tot = smallp.tile([128, 1], F32, tag="tot")
                    nc.vector.tensor_reduce(tot, acc[:, i, :], AX.X, ALU.add)
                    nc.scalar.activation(lse_sb[:, i:i + 1], tot, AF.Ln)
                nc.sync.dma_start(out=lse_d.rearrange("(i p) -> p i", p=128),
                                  in_=lse_sb)

    nc.finalize()
    return nc


def kernel(x, y, tok_emb, pos_emb, ln1_w, ln1_b, attn_w, attn_b, proj_w, proj_b,
           ln2_w, ln2_b, ffn_w1, ffn_b1, ffn_w2, ffn_b2, lnf_w, lnf_b,
           head_w, head_b):
    x = np.asarray(x)
    y = np.asarray(y)
    f = lambda a: np.asarray(a, np.float32)
    tok_emb, pos_emb = f(tok_emb), f(pos_emb)
    ln1_w, ln1_b, attn_w, attn_b = f(ln1_w), f(ln1_b), f(attn_w), f(attn_b)
    proj_w, proj_b, ln2_w, ln2_b = f(proj_w), f(proj_b), f(ln2_w), f(ln2_b)
    ffn_w1, ffn_b1, ffn_w2, ffn_b2 = f(ffn_w1), f(ffn_b1), f(ffn_w2), f(ffn_b2)
    lnf_w, lnf_b, head_w, head_b = f(lnf_w), f(lnf_b), f(head_w), f(head_b)

    bf = lambda a: np.ascontiguousarray(a.astype(ml_dtypes.bfloat16))

    # ---- host preprocessing: embeddings + LN folding + head-dim padding ----
    h0 = tok_emb[x] + pos_emb[None, :, :]                      # [B,T,C] f32

    WQKV = np.zeros((L, C, 2 * CP + C), np.float32)
    BQK = np.zeros((L, 2 * CP), np.float32)
    WPROJ = np.zeros((L, CP, C), np.float32)
    PB = np.zeros((L, C), np.float32)
    W1 = np.zeros((L, C, F), np.float32)
    B1 = np.zeros((L, F), np.float32)
    for l in range(L):
        aw = ln1_w[l][:, None] * attn_w[l]                     # [C, 3C]
        ab = attn_b[l] + ln1_b[l] @ attn_w[l]                  # [3C]
        for hh in range(H):
            WQKV[l, :, hh * 128:hh * 128 + D] = aw[:, hh * D:(hh + 1) * D]
            WQKV[l, :, CP + hh * 128:CP + hh * 128 + D] = aw[:, C + hh * D:C + (hh + 1) * D]
            BQK[l, hh * 128:hh * 128 + D] = ab[hh * D:(hh + 1) * D]
            BQK[l, CP + hh * 128:CP + hh * 128 + D] = ab[C + hh * D:C + (hh + 1) * D]
            WPROJ[l, hh * 128:hh * 128 + D, :] = proj_w[l][hh * D:(hh + 1) * D, :]
        WQKV[l, :, 2 * CP:] = aw[:, 2 * C:]                    # v (unpadded)
        PB[l] = proj_b[l] + ab[2 * C:] @ proj_w[l]             # v-bias folded
        W1[l] = ln2_w[l][:, None] * ffn_w1[l]
        B1[l] = ffn_b1[l] + ln2_b[l] @ ffn_w1[l]
    WHE = lnf_w[:, None] * head_w                              # [C, V]
    HB = head_b + lnf_b @ head_w                               # [V]

    has_qkbias = bool(np.any(BQK))
    has_pbias = bool(np.any(PB))
    has_b2bias = bool(np.any(ffn_b2))
    has_hbias = bool(np.any(HB))
    if has_hbias:
        WHEAD = np.zeros((896, V), np.float32)
        WHEAD[:C] = WHE
        WHEAD[C] = HB
    else:
        WHEAD = WHE

    debug = os.environ.get("KERNEL_DEBUG") == "1"
    key = (has_qkbias, has_pbias, has_b2bias, has_hbias, debug)
    if key not in _cache:
        _cache[key] = _build_program(*key[:4], debug=debug)
    nc = _cache[key]

    shared = {
        "wqkv": bf(WQKV), "wproj": bf(WPROJ), "w1": bf(W1), "w2": bf(ffn_w2),
        "b1": np.ascontiguousarray(B1), "whead": bf(WHEAD),
        "idn": np.eye(128, dtype=ml_dtypes.bfloat16),
        "ut": np.triu(np.ones((128, 128), ml_dtypes.bfloat16)),
    }
    if has_qkbias:
        shared["bqk"] = np.ascontiguousarray(BQK)
    if has_pbias:
        shared["pb"] = np.ascontiguousarray(PB)
    if has_b2bias:
        shared["b2"] = np.ascontiguousarray(ffn_b2)

    in_maps = [dict(shared, h0=np.ascontiguousarray(h0[c])) for c in range(B)]

    trace = os.environ.get("KERNEL_TRACE") == "1"
    res = run_bass_kernel_spmd(nc, in_maps, core_ids=list(range(NCORES)),
                               trace=trace)
    if trace and res.exec_time_ns is not None:
        print(f"HW exec time: {res.exec_time_ns} ns")
        kernel.last_exec_time_ns = res.exec_time_ns

    if debug:
        kernel.debug_out = res.results[0]
    logits = np.stack([res.results[c]["logits"] for c in range(B)])   # [B,T,V]
    lse = np.stack([res.results[c]["lse"] for c in range(B)])          # [B,T]
    ly = np.take_along_axis(logits.reshape(B * T, V),
                            y.reshape(B * T, 1).astype(np.int64), axis=1)[:, 0]
    loss = np.float32(np.mean(lse.reshape(B * T) - ly))
    return logits, loss


# revision 14
# speedup vs baseline: 1.2830x; 1.1165x over previous
"""Bass/Trainium2 kernel for nn_BigramLanguageModel (6-layer GPT, B=8,T=1024,C=768,V=32000).

Strategy: data-parallel over batch across the 8 NeuronCores (one batch element
per core, no collectives). Each core runs the full transformer + lm_head +
logsumexp for its sequence; the host folds LN weights/biases into adjacent
matmul weights, gathers embeddings, and combines per-core loss partials.
"""

import os
import numpy as np
import ml_dtypes

import concourse.bass as bass
import concourse.mybir as mybir
import concourse.tile as tile
from concourse import bacc
from concourse.bass_utils import run_bass_kernel_spmd

_orig_get_act_tables = bacc.get_activation_tables

def _steered_act_tables(arch):
    tabs = {k: set(v) for k, v in _orig_get_act_tables(arch).items()}
    combined = tabs.get("natural_log_exp_and_others")
    if combined and AF.Exp in combined and AF.Ln in combined:
        for name, fns in tabs.items():
            if name != "natural_log_exp_and_others":
                fns.discard(AF.Exp)
                fns.discard(AF.Ln)
    return tabs

bacc.get_activation_tables = _steered_act_tables

F32 = mybir.dt.float32
BF16 = mybir.dt.bfloat16
AF = mybir.ActivationFunctionType
ALU = mybir.AluOpType
AX = mybir.AxisListType

V = 32000
T = 1024
C = 768
L = 6
H = 8
B = 8
D = 96          # real head dim
DP = 128        # padded head dim
CP = H * DP     # 1024, padded attn-concat dim
F = 4 * C       # 3072
EPS = 1e-5
NCORES = 8
NT = T // 128   # 8 token tiles
NCO = C // 128  # 6 feature tiles
NV = (V + 511) // 512  # 63 vocab chunks (62 full + 1 of 256)

_cache = {}


def _ln_into(nc, pools, h_sb, hnT, idn, i):
    """LayerNorm (stats+normalize, weights pre-folded) of token tile i of h_sb
    into transposed hnT[:, :, i*128:(i+1)*128] (bf16)."""
    small = pools["small"]
    psA = pools["psA"]
    stats = small.tile([128, 3, 6], F32, tag="stats")
    for s in range(3):
        nc.vector.bn_stats(out=stats[:, s, :], in_=h_sb[:, i, s * 256:(s + 1) * 256])
    mv = small.tile([128, 2], F32, tag="mv")
    nc.vector.bn_aggr(out=mv, in_=stats)
    # rstd = exp(-0.5*ln(var+eps)); ln/exp share one ACT table set
    rstd = small.tile([128, 1], F32, tag="rstd")
    nc.scalar.activation(rstd, mv[:, 1:2], AF.Ln, bias=pools["eps"])
    nc.scalar.activation(rstd, rstd, AF.Exp, scale=-0.5)
    nm = small.tile([128, 1], F32, tag="nm")
    nc.vector.tensor_scalar(nm, mv[:, 0:1], rstd, -1.0, ALU.mult, ALU.mult)
    hn_b = small.tile([128, 768], BF16, tag="hn_b")
    nc.scalar.activation(hn_b, h_sb[:, i, :], AF.Identity, bias=nm, scale=rstd)
    pst = psA.tile([128, 768], BF16, tag="pst")
    for co in range(NCO):
        nc.tensor.transpose(pst[:, co * 128:(co + 1) * 128],
                            hn_b[:, co * 128:(co + 1) * 128], idn)
    nc.vector.tensor_copy(
        out=hnT[:, 0:NCO, i * 128:(i + 1) * 128],
        in_=pst.rearrange("p (o t) -> p o t", t=128))


def _build_program(has_qkbias, has_pbias, has_b2bias, has_hbias, debug=False):
    KH = 896 if has_hbias else 768
    KHO = KH // 128
    nc = bacc.Bacc()
    dbg = {}
    if debug:
        dbg["hnT"] = nc.declare_dram_parameter("dbg_hnT", [128, NCO, T], BF16, True)
        dbg["qkh"] = nc.declare_dram_parameter("dbg_qkh", [128, 2, T], BF16, True)
        dbg["vaug"] = nc.declare_dram_parameter("dbg_vaug", [128, NT, H, 98], BF16, True)
        dbg["pt"] = nc.declare_dram_parameter("dbg_pt", [128, T], BF16, True)
        dbg["rsc"] = nc.declare_dram_parameter("dbg_rsc", [128, T], F32, True)
        dbg["rep"] = nc.declare_dram_parameter("dbg_rep", [128, T], BF16, True)
        dbg["attnT"] = nc.declare_dram_parameter("dbg_attnT", [128, H, T], BF16, True)
        dbg["h1"] = nc.declare_dram_parameter("dbg_h1", [128, NT, C], F32, True)

    h0_d = nc.declare_dram_parameter("h0", [T, C], F32, False)
    wqkv_d = nc.declare_dram_parameter("wqkv", [L, C, 2 * CP + C], BF16, False)
    wproj_d = nc.declare_dram_parameter("wproj", [L, CP, C], BF16, False)
    w1_d = nc.declare_dram_parameter("w1", [L, C, F], BF16, False)
    w2_d = nc.declare_dram_parameter("w2", [L, F, C], BF16, False)
    b1_d = nc.declare_dram_parameter("b1", [L, F], F32, False)
    idn_d = nc.declare_dram_parameter("idn", [128, 128], BF16, False)
    ut_d = nc.declare_dram_parameter("ut", [128, 128], BF16, False)
    whead_d = nc.declare_dram_parameter("whead", [KH, V], BF16, False)
    if has_qkbias:
        bqk_d = nc.declare_dram_parameter("bqk", [L, 2 * CP], F32, False)
    if has_pbias:
        pb_d = nc.declare_dram_parameter("pb", [L, C], F32, False)
    if has_b2bias:
        b2_d = nc.declare_dram_parameter("b2", [L, C], F32, False)
    logits_d = nc.declare_dram_parameter("logits", [T, V], F32, True)
    lse_d = nc.declare_dram_parameter("lse", [T], F32, True)

    with tile.TileContext(nc) as tc:
        import contextlib
        stack = contextlib.ExitStack()
        with stack:
            const = stack.enter_context(tc.tile_pool(name="const", bufs=1))
            hnp = stack.enter_context(tc.tile_pool(name="hnp", bufs=1))
            smallp = stack.enter_context(tc.tile_pool(name="small", bufs=4))

            idn = const.tile([128, 128], BF16)
            nc.sync.dma_start(out=idn, in_=idn_d[:, :])
            ut = const.tile([128, 128], BF16)
            nc.sync.dma_start(out=ut, in_=ut_d[:, :])
            h_sb = const.tile([128, NT, C], F32)
            nc.sync.dma_start(out=h_sb, in_=h0_d.rearrange("(i p) c -> p i c", p=128))
            eps_sb = const.tile([128, 1], F32)
            nc.vector.memset(eps_sb, EPS)
            ones96 = const.tile([128, 96], BF16)
            nc.vector.memset(ones96, 1.0)

            # rolling per-layer weight pools: wqkv prefetched one layer ahead
            wq_ctx = {}
            wq_sb = {}

            def open_wqkv(l):
                ctx = tc.tile_pool(name=f"wqkv{l}", bufs=1, side="right")
                pool = ctx.__enter__()
                t = pool.tile([128, NCO, 2 * CP + C], BF16)
                nc.sync.dma_start(
                    out=t, in_=wqkv_d[l].rearrange("(o p) m -> p o m", p=128))
                wq_ctx[l] = ctx
                wq_sb[l] = t

            open_wqkv(0)

            hnA = hnp.tile([128, NCO, T], BF16, tag="hnA")
            with tc.tile_pool(name="psA0", bufs=2, space="PSUM") as psA0:
                pools = {"small": smallp, "psA": psA0, "eps": eps_sb}
                for i in range(NT):
                    _ln_into(nc, pools, h_sb, hnA, idn, i)

            for l in range(L):
                wqkv_sb = wq_sb[l]
                pctx = tc.tile_pool(name=f"wproj{l}", bufs=1, side="right")
                ppool = pctx.__enter__()
                wproj_sb = ppool.tile([128, H, C], BF16)
                nc.sync.dma_start(
                    out=wproj_sb, in_=wproj_d[l].rearrange("(o p) n -> p o n", p=128))
                if has_qkbias:
                    bqk_sb = smallp.tile([128, 16], F32, tag="bqk")
                    nc.sync.dma_start(
                        out=bqk_sb, in_=bqk_d[l].rearrange("(o p) -> p o", p=128))

                hnT = hnA

                # ---- Phase B: V (token-major) + V_aug ----
                at_ctx = tc.tile_pool(name=f"atT{l}", bufs=1, side="right")
                atpool = at_ctx.__enter__()
                attnT = atpool.tile([128, H, T], BF16)
                nc.gpsimd.memset(attnT[96:128, :, :], 0.0)
                with tc.tile_pool(name=f"vau{l}", bufs=1) as vpool, \
                     tc.tile_pool(name=f"att{l}", bufs=3) as apool:
                    vaug = vpool.tile([128, NT, H, 98], BF16)
                    nc.gpsimd.memset(vaug[:, :, :, 96:97], 1.0)
                    with tc.tile_pool(name=f"psB{l}", bufs=2, space="PSUM") as psB:
                        for j in range(NT):
                            vps = psB.tile([128, C], F32, tag="vps")
                            for n0, nw in ((0, 512), (512, 256)):
                                for co in range(NCO):
                                    nc.tensor.matmul(
                                        vps[:, n0:n0 + nw],
                                        lhsT=hnT[:, co, j * 128:(j + 1) * 128],
                                        rhs=wqkv_sb[:, co, 2 * CP + n0:2 * CP + n0 + nw],
                                        start=(co == 0), stop=(co == NCO - 1))
                            nc.vector.tensor_copy(
                                out=vaug[:, j, :, 0:96],
                                in_=vps.rearrange("p (h d) -> p h d", d=96))

                    if debug and l == 0:
                        nc.sync.dma_start(out=dbg["hnT"][:, :, :], in_=hnT)
                        nc.sync.dma_start(out=dbg["vaug"][:, :, :, :], in_=vaug)
                    # ---- Phase C: attention per head ----
                    psC_ctx = tc.tile_pool(name=f"psC{l}", bufs=4, space="PSUM")
                    psC = psC_ctx.__enter__()
                    psO_ctx = tc.tile_pool(name=f"psO{l}", bufs=2, space="PSUM")
                    psO = psO_ctx.__enter__()

                    def emit_norm(hh, ops, l=l):
                        rs = apool.tile([128, T], F32, tag="rs")
                        nc.scalar.activation(rs[96:97, :], ops[96:97, :], AF.Ln)
                        rsb = apool.tile([128, T], BF16, tag="rsb")
                        nc.scalar.activation(rsb[96:97, :], rs[96:97, :], AF.Exp,
                                             scale=-1.0)
                        rep = apool.tile([128, T], BF16, tag="rep")
                        for c0 in (0, 512):
                            repp = psC.tile([128, 512], F32, tag="big")
                            nc.tensor.matmul(
                                repp[0:96, :],
                                lhsT=ones96[96:97, :],
                                rhs=rsb[96:97, c0:c0 + 512],
                                start=True, stop=True, tile_position=(96, 0))
                            nc.vector.tensor_copy(out=rep[0:96, c0:c0 + 512],
                                                  in_=repp[0:96, :])
                        nc.vector.tensor_tensor(
                            out=attnT[0:96, hh, :], in0=ops[0:96, :],
                            in1=rep[0:96, :], op=ALU.mult)
                        if debug and l == 0 and hh == 0:
                            osc = apool.tile([128, T], F32, tag="osc")
                            nc.vector.tensor_copy(out=osc, in_=ops)
                            nc.sync.dma_start(out=dbg["rsc"][:, :], in_=osc)
                            nc.sync.dma_start(out=dbg["rep"][:, :], in_=rep)

                    pending = None
                    for hh in range(H):
                        qkh = apool.tile([128, 2, T], BF16, tag="qkh")
                        for qk in range(2):
                            for n0 in (0, 512):
                                ps = psC.tile([128, 512], F32, tag="big")
                                for co in range(NCO):
                                    nc.tensor.matmul(
                                        ps,
                                        lhsT=wqkv_sb[:, co,
                                                     qk * CP + hh * 128:qk * CP + hh * 128 + 128],
                                        rhs=hnT[:, co, n0:n0 + 512],
                                        start=(co == 0), stop=(co == NCO - 1))
                                if has_qkbias:
                                    nc.scalar.activation(
                                        qkh[:, qk, n0:n0 + 512], ps, AF.Identity,
                                        bias=bqk_sb[:, qk * 8 + hh:qk * 8 + hh + 1])
                                else:
                                    nc.vector.tensor_copy(
                                        out=qkh[:, qk, n0:n0 + 512], in_=ps)

                        if pending is not None:
                            emit_norm(*pending)
                        ops = psO.tile([128, T], F32, tag="ops")
                        for j in range(NT):
                            q0 = j * 128
                            pt = apool.tile([128, T], BF16, tag="pt")
                            for c0 in range(q0 // 512 * 512, T, 512):
                                a = max(q0, c0)
                                st = psC.tile([128, 512], F32, tag="big")
                                nc.tensor.matmul(
                                    st[:, 0:c0 + 512 - a],
                                    lhsT=qkh[:, 1, q0:q0 + 128],
                                    rhs=qkh[:, 0, a:c0 + 512],
                                    start=True, stop=True)
                                nc.scalar.activation(pt[:, a:c0 + 512],
                                                     st[:, 0:c0 + 512 - a], AF.Exp,
                                                     scale=float(D) ** -0.5)
                            nc.vector.tensor_tensor(
                                out=pt[:, q0:q0 + 128], in0=pt[:, q0:q0 + 128],
                                in1=ut, op=ALU.mult)
                            if debug and l == 0 and hh == 0 and j == 0:
                                nc.sync.dma_start(out=dbg["pt"][:, :], in_=pt)
                            for c0 in range(0, T, 512):
                                if c0 + 512 <= q0:
                                    continue
                                a = max(q0, c0)
                                nc.tensor.matmul(
                                    ops[0:97, a:c0 + 512],
                                    lhsT=vaug[:, j, hh, 0:97],
                                    rhs=pt[:, a:c0 + 512],
                                    start=(j == 0),
                                    stop=(j == min(7, (c0 + 511) // 128)))
                        if debug and l == 0 and hh == 0:
                            nc.sync.dma_start(out=dbg["qkh"][:, :, :], in_=qkh)
                        pending = (hh, ops)
                    emit_norm(*pending)
                    psO_ctx.__exit__(None, None, None)
                    psC_ctx.__exit__(None, None, None)
                    if debug and l == 0:
                        nc.sync.dma_start(out=dbg["attnT"][:, :, :], in_=attnT)

                # ---- Phase D: proj + residual + LN2 ----
                hnB = hnp.tile([128, NCO, T], BF16, tag="hnB")
                with tc.tile_pool(name=f"psD{l}", bufs=2, space="PSUM") as psD, \
                     tc.tile_pool(name=f"psDE{l}", bufs=2, space="PSUM") as psDE:
                    pools = {"small": smallp, "psA": psDE, "eps": eps_sb}
                    if has_pbias:
                        pbrow = smallp.tile([128, C], F32, tag="pbrow")
                        nc.sync.dma_start(out=pbrow[0:1, :], in_=pb_d[l][None, :])
                        pbrep = smallp.tile([128, C], F32, tag="pbrep")
                        nc.gpsimd.partition_broadcast(pbrep, pbrow[0:1, :])
                    for i in range(NT):
                        pp = psD.tile([128, C], F32, tag="pp")
                        for n0, nw in ((0, 512), (512, 256)):
                            for kt in range(H):
                                nc.tensor.matmul(
                                    pp[:, n0:n0 + nw],
                                    lhsT=attnT[:, kt, i * 128:(i + 1) * 128],
                                    rhs=wproj_sb[:, kt, n0:n0 + nw],
                                    start=(kt == 0), stop=(kt == H - 1))
                        nc.vector.tensor_add(out=h_sb[:, i, :], in0=h_sb[:, i, :],
                                             in1=pp)
                        if has_pbias:
                            nc.vector.tensor_add(out=h_sb[:, i, :],
                                                 in0=h_sb[:, i, :], in1=pbrep)
                        _ln_into(nc, pools, h_sb, hnB, idn, i)
                if debug and l == 0:
                    nc.sync.dma_start(out=dbg["h1"][:, :, :], in_=h_sb)
                at_ctx.__exit__(None, None, None)
                pctx.__exit__(None, None, None)
                wq_ctx[l].__exit__(None, None, None)

                # ---- Phase F: MLP (+ next layer's LN1) ----
                wctx = tc.tile_pool(name=f"wffn{l}", bufs=1)
                wpool = wctx.__enter__()
                w1_sb = wpool.tile([128, NCO, F], BF16, tag="w1")
                nc.sync.dma_start(
                    out=w1_sb, in_=w1_d[l].rearrange("(o p) m -> p o m", p=128))
                w2_sb = wpool.tile([128, F // 128, C], BF16, tag="w2")
                nc.sync.dma_start(
                    out=w2_sb, in_=w2_d[l].rearrange("(o p) n -> p o n", p=128))
                b1_sb = smallp.tile([128, F // 128], F32, tag="b1")
                nc.sync.dma_start(
                    out=b1_sb, in_=b1_d[l].rearrange("(o p) -> p o", p=128))
                if has_b2bias:
                    b2row = smallp.tile([128, C], F32, tag="b2row")
                    nc.sync.dma_start(out=b2row[0:1, :], in_=b2_d[l][None, :])
                    b2rep = smallp.tile([128, C], F32, tag="b2rep")
                    nc.gpsimd.partition_broadcast(b2rep, b2row[0:1, :])

                if l + 1 < L:
                    hnA = hnp.tile([128, NCO, T], BF16, tag="hnA")
                with tc.tile_pool(name=f"fc{l}", bufs=1) as fcp, \
                     tc.tile_pool(name=f"psF1{l}", bufs=2, space="PSUM") as psF1, \
                     tc.tile_pool(name=f"psF2{l}", bufs=2, space="PSUM") as psF2, \
                     tc.tile_pool(name=f"psFA{l}", bufs=2, space="PSUM") as psFA:
                    pools = {"small": smallp, "psA": psFA, "eps": eps_sb}
                    for quarter in range(4):
                        t0 = quarter * 256
                        fc1 = fcp.tile([128, F // 128, 256], BF16, tag="fc1")
                        for m in range(F // 128):
                            fp = psF1.tile([128, 256], F32, tag="fp")
                            for co in range(NCO):
                                nc.tensor.matmul(
                                    fp, lhsT=w1_sb[:, co, m * 128:(m + 1) * 128],
                                    rhs=hnB[:, co, t0:t0 + 256],
                                    start=(co == 0), stop=(co == NCO - 1))
                            nc.scalar.activation(fc1[:, m, :], fp, AF.Relu,
                                                 bias=b1_sb[:, m:m + 1])
                        # prefetch next layer's qkv weights
                        if quarter == 0 and l + 1 < L:
                            open_wqkv(l + 1)
                        for ii in range(2):
                            i = quarter * 2 + ii
                            p2 = psF2.tile([128, C], F32, tag="p2")
                            for n0, nw in ((0, 512), (512, 256)):
                                for kt in range(F // 128):
                                    nc.tensor.matmul(
                                        p2[:, n0:n0 + nw],
                                        lhsT=fc1[:, kt, ii * 128:(ii + 1) * 128],
                                        rhs=w2_sb[:, kt, n0:n0 + nw],
                                        start=(kt == 0), stop=(kt == F // 128 - 1))
                            nc.vector.tensor_add(out=h_sb[:, i, :],
                                                 in0=h_sb[:, i, :], in1=p2)
                            if has_b2bias:
                                nc.vector.tensor_add(out=h_sb[:, i, :],
                                                     in0=h_sb[:, i, :], in1=b2rep)
                            if l + 1 < L:
                                _ln_into(nc, pools, h_sb, hnA, idn, i)
                wctx.__exit__(None, None, None)

            # ---- Phase G: final LN + lm_head + logsumexp ----
            hfp = stack.enter_context(tc.tile_pool(name="hfp", bufs=1))
            hfT = hfp.tile([128, KHO, T], BF16, tag="hnT2")
            with tc.tile_pool(name="psLNf", bufs=2, space="PSUM") as psA:
                pools = {"small": smallp, "psA": psA, "eps": eps_sb}
                for i in range(NT):
                    _ln_into(nc, pools, h_sb, hfT, idn, i)
            if has_hbias:
                nc.vector.memset(hfT[:, 6, :], 0.0)
                nc.vector.memset(hfT[0:1, 6, :], 1.0)

            acc = hfp.tile([128, NT, NV], F32)
            with tc.tile_pool(name="whp", bufs=3) as whp, \
                 tc.tile_pool(name="lgp", bufs=6) as lgp, \
                 tc.tile_pool(name="exg", bufs=2) as exg, \
                 tc.tile_pool(name="psG", bufs=6, space="PSUM") as psG:
                for n in range(NV):
                    n0 = n * 512
                    nw = min(512, V - n0)
                    wh = whp.tile([128, KHO, 512], BF16, tag="wh")
                    nc.sync.dma_start(
                        out=wh[:, :, 0:nw],
                        in_=whead_d[:, n0:n0 + nw].rearrange("(o p) m -> p o m", p=128))
                    for i in range(NT):
                        lp = psG.tile([128, 512], F32, tag="lp")
                        for co in range(KHO):
                            nc.tensor.matmul(
                                lp[:, 0:nw], lhsT=hfT[:, co, i * 128:(i + 1) * 128],
                                rhs=wh[:, co, 0:nw],
                                start=(co == 0), stop=(co == KHO - 1))
                        lg = lgp.tile([128, 512], F32, tag="lg")
                        nc.vector.tensor_copy(out=lg[:, 0:nw], in_=lp[:, 0:nw])
                        nc.sync.dma_start(
                            out=logits_d[i * 128:(i + 1) * 128, n0:n0 + nw],
                            in_=lg[:, 0:nw])
                        ex = exg.tile([128, 512], BF16, tag="ex")
                        nc.scalar.activation(ex[:, 0:nw], lg[:, 0:nw], AF.Exp,
                                             accum_out=acc[:, i, n:n + 1])
                lse_sb = hfp.tile([128, NT], F32)
                for i in range(NT):
                    tot = smallp.tile([128, 1], F32, tag="tot")
                    nc.vector.tensor_reduce(tot, acc[:, i, :], AX.X, ALU.add)
                    nc.scalar.activation(lse_sb[:, i:i + 1], tot, AF.Ln)
                nc.sync.dma_start(out=lse_d.rearrange("(i p) -> p i", p=128),
                                  in_=lse_sb)

    nc.finalize()
    return nc


def kernel(x, y, tok_emb, pos_emb, ln1_w, ln1_b, attn_w, attn_b, proj_w, proj_b,
           ln2_w, ln2_b, ffn_w1, ffn_b1, ffn_w2, ffn_b2, lnf_w, lnf_b,
           head_w, head_b):
    x = np.asarray(x)
    y = np.asarray(y)
    f = lambda a: np.asarray(a, np.float32)
    tok_emb, pos_emb = f(tok_emb), f(pos_emb)
    ln1_w, ln1_b, attn_w, attn_b = f(ln1_w), f(ln1_b), f(attn_w), f(attn_b)
    proj_w, proj_b, ln2_w, ln2_b = f(proj_w), f(proj_b), f(ln2_w), f(ln2_b)
    ffn_w1, ffn_b1, ffn_w2, ffn_b2 = f(ffn_w1), f(ffn_b1), f(ffn_w2), f(ffn_b2)
    lnf_w, lnf_b, head_w, head_b = f(lnf_w), f(lnf_b), f(head_w), f(head_b)

    bf = lambda a: np.ascontiguousarray(a.astype(ml_dtypes.bfloat16))

    # ---- host preprocessing: embeddings + LN folding + head-dim padding ----
    h0 = tok_emb[x] + pos_emb[None, :, :]                      # [B,T,C] f32

    WQKV = np.zeros((L, C, 2 * CP + C), np.float32)
    BQK = np.zeros((L, 2 * CP), np.float32)
    WPROJ = np.zeros((L, CP, C), np.float32)
    PB = np.zeros((L, C), np.float32)
    W1 = np.zeros((L, C, F), np.float32)
    B1 = np.zeros((L, F), np.float32)
    for l in range(L):
        aw = ln1_w[l][:, None] * attn_w[l]                     # [C, 3C]
        ab = attn_b[l] + ln1_b[l] @ attn_w[l]                  # [3C]
        for hh in range(H):
            WQKV[l, :, hh * 128:hh * 128 + D] = aw[:, hh * D:(hh + 1) * D]
            WQKV[l, :, CP + hh * 128:CP + hh * 128 + D] = aw[:, C + hh * D:C + (hh + 1) * D]
            BQK[l, hh * 128:hh * 128 + D] = ab[hh * D:(hh + 1) * D]
            BQK[l, CP + hh * 128:CP + hh * 128 + D] = ab[C + hh * D:C + (hh + 1) * D]
            WPROJ[l, hh * 128:hh * 128 + D, :] = proj_w[l][hh * D:(hh + 1) * D, :]
        WQKV[l, :, 2 * CP:] = aw[:, 2 * C:]                    # v (unpadded)
        PB[l] = proj_b[l] + ab[2 * C:] @ proj_w[l]             # v-bias folded
        W1[l] = ln2_w[l][:, None] * ffn_w1[l]
        B1[l] = ffn_b1[l] + ln2_b[l] @ ffn_w1[l]
    WHE = lnf_w[:, None] * head_w                              # [C, V]
    HB = head_b + lnf_b @ head_w                               # [V]

    has_qkbias = bool(np.any(BQK))
    has_pbias = bool(np.any(PB))
    has_b2bias = bool(np.any(ffn_b2))
    has_hbias = bool(np.any(HB))
    if has_hbias:
        WHEAD = np.zeros((896, V), np.float32)
        WHEAD[:C] = WHE
        WHEAD[C] = HB
    else:
        WHEAD = WHE

    debug = os.environ.get("KERNEL_DEBUG") == "1"
    key = (has_qkbias, has_pbias, has_b2bias, has_hbias, debug)
    if key not in _cache:
        _cache[key] = _build_program(*key[:4], debug=debug)
    nc = _cache[key]

    shared = {
        "wqkv": bf(WQKV), "wproj": bf(WPROJ), "w1": bf(W1), "w2": bf(ffn_w2),
        "b1": np.ascontiguousarray(B1), "whead": bf(WHEAD),
        "idn": np.eye(128, dtype=ml_dtypes.bfloat16),
        "ut": np.triu(np.ones((128, 128), ml_dtypes.bfloat16)),
    }
    if has_qkbias:
        shared["bqk"] = np.ascontiguousarray(BQK)
    if has_pbias:
        shared["pb"] = np.ascontiguousarray(PB)
    if has_b2bias:
        shared["b2"] = np.ascontiguousarray(ffn_b2)

    in_maps = [dict(shared, h0=np.ascontiguousarray(h0[c])) for c in range(B)]

    trace = os.environ.get("KERNEL_TRACE") == "1"
    res = run_bass_kernel_spmd(nc, in_maps, core_ids=list(range(NCORES)),
                               trace=trace)
    if trace and res.exec_time_ns is not None:
        print(f"HW exec time: {res.exec_time_ns} ns")
        kernel.last_exec_time_ns = res.exec_time_ns

    if debug:
        kernel.debug_out = res.results[0]
    logits = np.stack([res.results[c]["logits"] for c in range(B)])   # [B,T,V]
    lse = np.stack([res.results[c]["lse"] for c in range(B)])          # [B,T]
    ly = np.take_along_axis(logits.reshape(B * T, V),
                            y.reshape(B * T, 1).astype(np.int64), axis=1)[:, 0]
    loss = np.float32(np.mean(lse.reshape(B * T) - ly))
    return logits, loss


# revision 17
# speedup vs baseline: 1.2936x; 1.0082x over previous
"""Bass/Trainium2 kernel for nn_BigramLanguageModel (6-layer GPT, B=8,T=1024,C=768,V=32000).

Strategy: data-parallel over batch across the 8 NeuronCores (one batch element
per core, no collectives). Each core runs the full transformer + lm_head +
logsumexp for its sequence; the host folds LN weights/biases into adjacent
matmul weights, gathers embeddings, and combines per-core loss partials.
"""

import os
import numpy as np
import ml_dtypes

import concourse.bass as bass
import concourse.mybir as mybir
import concourse.tile as tile
from concourse import bacc
from concourse.bass_utils import run_bass_kernel_spmd

_orig_get_act_tables = bacc.get_activation_tables

def _steered_act_tables(arch):
    tabs = {k: set(v) for k, v in _orig_get_act_tables(arch).items()}
    combined = tabs.get("natural_log_exp_and_others")
    if combined and AF.Exp in combined and AF.Ln in combined:
        for name, fns in tabs.items():
            if name != "natural_log_exp_and_others":
                fns.discard(AF.Exp)
                fns.discard(AF.Ln)
    return tabs

bacc.get_activation_tables = _steered_act_tables

F32 = mybir.dt.float32
BF16 = mybir.dt.bfloat16
AF = mybir.ActivationFunctionType
ALU = mybir.AluOpType
AX = mybir.AxisListType

V = 32000
T = 1024
C = 768
L = 6
H = 8
B = 8
D = 96          # real head dim
DP = 128        # padded head dim
CP = H * DP     # 1024, padded attn-concat dim
F = 4 * C       # 3072
EPS = 1e-5
NCORES = 8
NT = T // 128   # 8 token tiles
NCO = C // 128  # 6 feature tiles
NV = (V + 511) // 512  # 63 vocab chunks (62 full + 1 of 256)

_cache = {}


def _ln_into(nc, pools, h_sb, hnT, idn, i):
    """LayerNorm (stats+normalize, weights pre-folded) of token tile i of h_sb
    into transposed hnT[:, :, i*128:(i+1)*128] (bf16)."""
    small = pools["small"]
    psA = pools["psA"]
    stats = small.tile([128, 3, 6], F32, tag="stats")
    for s in range(3):
        nc.vector.bn_stats(out=stats[:, s, :], in_=h_sb[:, i, s * 256:(s + 1) * 256])
    mv = small.tile([128, 2], F32, tag="mv")
    nc.vector.bn_aggr(out=mv, in_=stats)
    # rstd = exp(-0.5*ln(var+eps)); ln/exp share one ACT table set
    rstd = small.tile([128, 1], F32, tag="rstd")
    nc.scalar.activation(rstd, mv[:, 1:2], AF.Ln, bias=pools["eps"])
    nc.scalar.activation(rstd, rstd, AF.Exp, scale=-0.5)
    nm = small.tile([128, 1], F32, tag="nm")
    nc.vector.tensor_scalar(nm, mv[:, 0:1], rstd, -1.0, ALU.mult, ALU.mult)
    hn_b = small.tile([128, 768], BF16, tag="hn_b")
    nc.scalar.activation(hn_b, h_sb[:, i, :], AF.Identity, bias=nm, scale=rstd)
    pst = psA.tile([128, 768], BF16, tag="pst")
    for co in range(NCO):
        nc.tensor.transpose(pst[:, co * 128:(co + 1) * 128],
                            hn_b[:, co * 128:(co + 1) * 128], idn)
    nc.vector.tensor_copy(
        out=hnT[:, 0:NCO, i * 128:(i + 1) * 128],
        in_=pst.rearrange("p (o t) -> p o t", t=128))


def _build_program(has_qkbias, has_pbias, has_b2bias, has_hbias, debug=False):
    KH = 896 if has_hbias else 768
    KHO = KH // 128
    nc = bacc.Bacc()
    dbg = {}
    if debug:
        dbg["hnT"] = nc.declare_dram_parameter("dbg_hnT", [128, NCO, T], BF16, True)
        dbg["qkh"] = nc.declare_dram_parameter("dbg_qkh", [128, 2, T], BF16, True)
        dbg["vaug"] = nc.declare_dram_parameter("dbg_vaug", [128, NT, H, 98], BF16, True)
        dbg["pt"] = nc.declare_dram_parameter("dbg_pt", [128, T], BF16, True)
        dbg["rsc"] = nc.declare_dram_parameter("dbg_rsc", [128, T], F32, True)
        dbg["rep"] = nc.declare_dram_parameter("dbg_rep", [128, T], BF16, True)
        dbg["attnT"] = nc.declare_dram_parameter("dbg_attnT", [128, H, T], BF16, True)
        dbg["h1"] = nc.declare_dram_parameter("dbg_h1", [128, NT, C], F32, True)

    h0_d = nc.declare_dram_parameter("h0", [T, C], F32, False)
    wqkv_d = nc.declare_dram_parameter("wqkv", [L, C, 2 * CP + C], BF16, False)
    wproj_d = nc.declare_dram_parameter("wproj", [L, CP, C], BF16, False)
    w1_d = nc.declare_dram_parameter("w1", [L, C, F], BF16, False)
    w2_d = nc.declare_dram_parameter("w2", [L, F, C], BF16, False)
    b1_d = nc.declare_dram_parameter("b1", [L, F], F32, False)
    idn_d = nc.declare_dram_parameter("idn", [128, 128], BF16, False)
    ut_d = nc.declare_dram_parameter("ut", [128, 128], BF16, False)
    whead_d = nc.declare_dram_parameter("whead", [KH, V], BF16, False)
    if has_qkbias:
        bqk_d = nc.declare_dram_parameter("bqk", [L, 2 * CP], F32, False)
    if has_pbias:
        pb_d = nc.declare_dram_parameter("pb", [L, C], F32, False)
    if has_b2bias:
        b2_d = nc.declare_dram_parameter("b2", [L, C], F32, False)
    logits_d = nc.declare_dram_parameter("logits", [T, V], F32, True)
    lse_d = nc.declare_dram_parameter("lse", [T], F32, True)

    with tile.TileContext(nc) as tc:
        import contextlib
        stack = contextlib.ExitStack()
        with stack:
            const = stack.enter_context(tc.tile_pool(name="const", bufs=1))
            hnp = stack.enter_context(tc.tile_pool(name="hnp", bufs=1))
            smallp = stack.enter_context(tc.tile_pool(name="small", bufs=4))

            idn = const.tile([128, 128], BF16)
            nc.sync.dma_start(out=idn, in_=idn_d[:, :])
            ut = const.tile([128, 128], BF16)
            nc.sync.dma_start(out=ut, in_=ut_d[:, :])
            h_sb = const.tile([128, NT, C], F32)
            nc.sync.dma_start(out=h_sb, in_=h0_d.rearrange("(i p) c -> p i c", p=128))
            eps_sb = const.tile([128, 1], F32)
            nc.vector.memset(eps_sb, EPS)
            ones96 = const.tile([128, 96], BF16)
            nc.vector.memset(ones96, 1.0)

            # rolling per-layer weight pools: wqkv prefetched one layer ahead
            wq_ctx = {}
            wq_sb = {}

            def open_wqkv(l):
                ctx = tc.tile_pool(name=f"wqkv{l}", bufs=1, side="right")
                pool = ctx.__enter__()
                t = pool.tile([128, NCO, 2 * CP + C], BF16)
                nc.sync.dma_start(
                    out=t, in_=wqkv_d[l].rearrange("(o p) m -> p o m", p=128))
                wq_ctx[l] = ctx
                wq_sb[l] = t

            open_wqkv(0)

            hnA = hnp.tile([128, NCO, T], BF16, tag="hnA")
            with tc.tile_pool(name="psA0", bufs=2, space="PSUM") as psA0:
                pools = {"small": smallp, "psA": psA0, "eps": eps_sb}
                for i in range(NT):
                    _ln_into(nc, pools, h_sb, hnA, idn, i)

            for l in range(L):
                wqkv_sb = wq_sb[l]
                pctx = tc.tile_pool(name=f"wproj{l}", bufs=1, side="right")
                ppool = pctx.__enter__()
                wproj_sb = ppool.tile([128, H, C], BF16)
                nc.sync.dma_start(
                    out=wproj_sb, in_=wproj_d[l].rearrange("(o p) n -> p o n", p=128))
                if has_qkbias:
                    bqk_sb = smallp.tile([128, 16], F32, tag="bqk")
                    nc.sync.dma_start(
                        out=bqk_sb, in_=bqk_d[l].rearrange("(o p) -> p o", p=128))

                hnT = hnA

                # ---- Phase B: V (token-major) + V_aug ----
                at_ctx = tc.tile_pool(name=f"atT{l}", bufs=1, side="right")
                atpool = at_ctx.__enter__()
                attnT = atpool.tile([128, H, T], BF16)
                nc.vector.memset(attnT[0:1, :, :], 0.0)
                nc.gpsimd.memset(attnT[96:128, :, :], 0.0)
                with tc.tile_pool(name=f"vau{l}", bufs=1) as vpool, \
                     tc.tile_pool(name=f"att{l}", bufs=3) as apool:
                    vaug = vpool.tile([128, NT, H, 98], BF16)
                    nc.gpsimd.memset(vaug[:, :, :, 0:1], 1.0)
                    with tc.tile_pool(name=f"psB{l}", bufs=2, space="PSUM") as psB:
                        for j in range(NT):
                            vps = psB.tile([128, C], F32, tag="vps")
                            for n0, nw in ((0, 512), (512, 256)):
                                for co in range(NCO):
                                    nc.tensor.matmul(
                                        vps[:, n0:n0 + nw],
                                        lhsT=hnT[:, co, j * 128:(j + 1) * 128],
                                        rhs=wqkv_sb[:, co, 2 * CP + n0:2 * CP + n0 + nw],
                                        start=(co == 0), stop=(co == NCO - 1))
                            nc.vector.tensor_copy(
                                out=vaug[:, j, :, 1:97],
                                in_=vps.rearrange("p (h d) -> p h d", d=96))

                    if debug and l == 0:
                        nc.sync.dma_start(out=dbg["hnT"][:, :, :], in_=hnT)
                        nc.sync.dma_start(out=dbg["vaug"][:, :, :, :], in_=vaug)
                    # ---- Phase C: attention per head ----
                    psC_ctx = tc.tile_pool(name=f"psC{l}", bufs=4, space="PSUM")
                    psC = psC_ctx.__enter__()
                    psO_ctx = tc.tile_pool(name=f"psO{l}", bufs=2, space="PSUM")
                    psO = psO_ctx.__enter__()

                    def emit_norm(hh, ops, l=l):
                        rs = apool.tile([128, T], F32, tag="rs")
                        nc.scalar.activation(rs[0:1, :], ops[0:1, :], AF.Ln)
                        rsb = apool.tile([128, T], BF16, tag="rsb")
                        nc.scalar.activation(rsb[0:1, :], rs[0:1, :], AF.Exp,
                                             scale=-1.0)
                        rep = apool.tile([128, T], BF16, tag="rep")
                        nc.gpsimd.partition_broadcast(rep[0:97, :], rsb[0:1, :])
                        nc.vector.tensor_tensor(
                            out=attnT[0:97, hh, :], in0=ops[0:97, :],
                            in1=rep[0:97, :], op=ALU.mult)
                        if debug and l == 0 and hh == 0:
                            osc = apool.tile([128, T], F32, tag="osc")
                            nc.vector.tensor_copy(out=osc, in_=ops)
                            nc.sync.dma_start(out=dbg["rsc"][:, :], in_=osc)
                            nc.sync.dma_start(out=dbg["rep"][:, :], in_=rep)

                    pending = None
                    for hh in range(H):
                        qkh = apool.tile([128, 2, T], BF16, tag="qkh")
                        for qk in range(2):
                            for n0 in (0, 512):
                                ps = psC.tile([128, 512], F32, tag="big")
                                for co in range(NCO):
                                    nc.tensor.matmul(
                                        ps,
                                        lhsT=wqkv_sb[:, co,
                                                     qk * CP + hh * 128:qk * CP + hh * 128 + 128],
                                        rhs=hnT[:, co, n0:n0 + 512],
                                        start=(co == 0), stop=(co == NCO - 1))
                                if has_qkbias:
                                    nc.scalar.activation(
                                        qkh[:, qk, n0:n0 + 512], ps, AF.Identity,
                                        bias=bqk_sb[:, qk * 8 + hh:qk * 8 + hh + 1])
                                else:
                                    nc.vector.tensor_copy(
                                        out=qkh[:, qk, n0:n0 + 512], in_=ps)

                        if pending is not None:
                            emit_norm(*pending)
                        ops = psO.tile([128, T], F32, tag="ops")
                        for j in range(NT):
                            q0 = j * 128
                            pt = apool.tile([128, T], BF16, tag="pt")
                            for c0 in range(q0 // 512 * 512, T, 512):
                                a = max(q0, c0)
                                st = psC.tile([128, 512], F32, tag="big")
                                nc.tensor.matmul(
                                    st[:, 0:c0 + 512 - a],
                                    lhsT=qkh[:, 1, q0:q0 + 128],
                                    rhs=qkh[:, 0, a:c0 + 512],
                                    start=True, stop=True)
                                nc.scalar.activation(pt[:, a:c0 + 512],
                                                     st[:, 0:c0 + 512 - a], AF.Exp,
                                                     scale=float(D) ** -0.5)
                            nc.vector.tensor_tensor(
                                out=pt[:, q0:q0 + 128], in0=pt[:, q0:q0 + 128],
                                in1=ut, op=ALU.mult)
                            if debug and l == 0 and hh == 0 and j == 0:
                                nc.sync.dma_start(out=dbg["pt"][:, :], in_=pt)
                            for c0 in range(0, T, 512):
                                if c0 + 512 <= q0:
                                    continue
                                a = max(q0, c0)
                                nc.tensor.matmul(
                                    ops[0:97, a:c0 + 512],
                                    lhsT=vaug[:, j, hh, 0:97],
                                    rhs=pt[:, a:c0 + 512],
                                    start=(j == 0),
                                    stop=(j == min(7, (c0 + 511) // 128)))
                        if debug and l == 0 and hh == 0:
                            nc.sync.dma_start(out=dbg["qkh"][:, :, :], in_=qkh)
                        pending = (hh, ops)
                    emit_norm(*pending)
                    psO_ctx.__exit__(None, None, None)
                    psC_ctx.__exit__(None, None, None)
                    if debug and l == 0:
                        nc.sync.dma_start(out=dbg["attnT"][:, :, :], in_=attnT)

                # ---- Phase D: proj + residual + LN2 ----
                hnB = hnp.tile([128, NCO, T], BF16, tag="hnB")
                with tc.tile_pool(name=f"psD{l}", bufs=2, space="PSUM") as psD, \
                     tc.tile_pool(name=f"psDE{l}", bufs=2, space="PSUM") as psDE:
                    pools = {"small": smallp, "psA": psDE, "eps": eps_sb}
                    if has_pbias:
                        pbrow = smallp.tile([128, C], F32, tag="pbrow")
                        nc.sync.dma_start(out=pbrow[0:1, :], in_=pb_d[l][None, :])
                        pbrep = smallp.tile([128, C], F32, tag="pbrep")
                        nc.gpsimd.partition_broadcast(pbrep, pbrow[0:1, :])
                    for i in range(NT):
                        pp = psD.tile([128, C], F32, tag="pp")
                        for n0, nw in ((0, 512), (512, 256)):
                            for kt in range(H):
                                nc.tensor.matmul(
                                    pp[:, n0:n0 + nw],
                                    lhsT=attnT[:, kt, i * 128:(i + 1) * 128],
                                    rhs=wproj_sb[:, kt, n0:n0 + nw],
                                    start=(kt == 0), stop=(kt == H - 1))
                        nc.vector.tensor_add(out=h_sb[:, i, :], in0=h_sb[:, i, :],
                                             in1=pp)
                        if has_pbias:
                            nc.vector.tensor_add(out=h_sb[:, i, :],
                                                 in0=h_sb[:, i, :], in1=pbrep)
                        _ln_into(nc, pools, h_sb, hnB, idn, i)
                if debug and l == 0:
                    nc.sync.dma_start(out=dbg["h1"][:, :, :], in_=h_sb)
                at_ctx.__exit__(None, None, None)
                pctx.__exit__(None, None, None)
                wq_ctx[l].__exit__(None, None, None)

                # ---- Phase F: MLP (+ next layer's LN1) ----
                wctx = tc.tile_pool(name=f"wffn{l}", bufs=1)
                wpool = wctx.__enter__()
                w1_sb = wpool.tile([128, NCO, F], BF16, tag="w1")
                nc.sync.dma_start(
                    out=w1_sb, in_=w1_d[l].rearrange("(o p) m -> p o m", p=128))
                w2_sb = wpool.tile([128, F // 128, C], BF16, tag="w2")
                nc.sync.dma_start(
                    out=w2_sb, in_=w2_d[l].rearrange("(o p) n -> p o n", p=128))
                b1_sb = smallp.tile([128, F // 128], F32, tag="b1")
                nc.sync.dma_start(
                    out=b1_sb, in_=b1_d[l].rearrange("(o p) -> p o", p=128))
                if has_b2bias:
                    b2row = smallp.tile([128, C], F32, tag="b2row")
                    nc.sync.dma_start(out=b2row[0:1, :], in_=b2_d[l][None, :])
                    b2rep = smallp.tile([128, C], F32, tag="b2rep")
                    nc.gpsimd.partition_broadcast(b2rep, b2row[0:1, :])

                if l + 1 < L:
                    hnA = hnp.tile([128, NCO, T], BF16, tag="hnA")
                with tc.tile_pool(name=f"fc{l}", bufs=1) as fcp, \
                     tc.tile_pool(name=f"psF1{l}", bufs=2, space="PSUM") as psF1, \
                     tc.tile_pool(name=f"psF2{l}", bufs=2, space="PSUM") as psF2, \
                     tc.tile_pool(name=f"psFA{l}", bufs=2, space="PSUM") as psFA:
                    pools = {"small": smallp, "psA": psFA, "eps": eps_sb}
                    for quarter in range(4):
                        t0 = quarter * 256
                        fc1 = fcp.tile([128, F // 128, 256], BF16, tag="fc1")
                        for m in range(F // 128):
                            fp = psF1.tile([128, 256], F32, tag="fp")
                            for co in range(NCO):
                                nc.tensor.matmul(
                                    fp, lhsT=w1_sb[:, co, m * 128:(m + 1) * 128],
                                    rhs=hnB[:, co, t0:t0 + 256],
                                    start=(co == 0), stop=(co == NCO - 1))
                            nc.scalar.activation(fc1[:, m, :], fp, AF.Relu,
                                                 bias=b1_sb[:, m:m + 1])
                        # prefetch next layer's qkv weights
                        if quarter == 0 and l + 1 < L:
                            open_wqkv(l + 1)
                        for ii in range(2):
                            i = quarter * 2 + ii
                            p2 = psF2.tile([128, C], F32, tag="p2")
                            for n0, nw in ((0, 512), (512, 256)):
                                for kt in range(F // 128):
                                    nc.tensor.matmul(
                                        p2[:, n0:n0 + nw],
                                        lhsT=fc1[:, kt, ii * 128:(ii + 1) * 128],
                                        rhs=w2_sb[:, kt, n0:n0 + nw],
                                        start=(kt == 0), stop=(kt == F // 128 - 1))
                            nc.vector.tensor_add(out=h_sb[:, i, :],
                                                 in0=h_sb[:, i, :], in1=p2)
                            if has_b2bias:
                                nc.vector.tensor_add(out=h_sb[:, i, :],
                                                     in0=h_sb[:, i, :], in1=b2rep)
                            if l + 1 < L:
                                _ln_into(nc, pools, h_sb, hnA, idn, i)
                wctx.__exit__(None, None, None)

            # ---- Phase G: final LN + lm_head + logsumexp ----
            hfp = stack.enter_context(tc.tile_pool(name="hfp", bufs=1))
            hfT = hfp.tile([128, KHO, T], BF16, tag="hnT2")
            with tc.tile_pool(name="psLNf", bufs=2, space="PSUM") as psA:
                pools = {"small": smallp, "psA": psA, "eps": eps_sb}
                for i in range(NT):
                    _ln_into(nc, pools, h_sb, hfT, idn, i)
            if has_hbias:
                nc.vector.memset(hfT[:, 6, :], 0.0)
                nc.vector.memset(hfT[0:1, 6, :], 1.0)

            acc = hfp.tile([128, NT, NV], F32)
            with tc.tile_pool(name="whp", bufs=3) as whp, \
                 tc.tile_pool(name="lgp", bufs=6) as lgp, \
                 tc.tile_pool(name="exg", bufs=2) as exg, \
                 tc.tile_pool(name="psG", bufs=6, space="PSUM") as psG:
                for n in range(NV):
                    n0 = n * 512
                    nw = min(512, V - n0)
                    wh = whp.tile([128, KHO, 512], BF16, tag="wh")
                    nc.sync.dma_start(
                        out=wh[:, :, 0:nw],
                        in_=whead_d[:, n0:n0 + nw].rearrange("(o p) m -> p o m", p=128))
                    for i in range(NT):
                        lp = psG.tile([128, 512], F32, tag="lp")
                        for co in range(KHO):
                            nc.tensor.matmul(
                                lp[:, 0:nw], lhsT=hfT[:, co, i * 128:(i + 1) * 128],
                                rhs=wh[:, co, 0:nw],
                                start=(co == 0), stop=(co == KHO - 1))
                        lg = lgp.tile([128, 512], F32, tag="lg")
                        nc.vector.tensor_copy(out=lg[:, 0:nw], in_=lp[:, 0:nw])
                        nc.sync.dma_start(
                            out=logits_d[i * 128:(i + 1) * 128, n0:n0 + nw],
                            in_=lg[:, 0:nw])
                        ex = exg.tile([128, 512], BF16, tag="ex")
                        nc.scalar.activation(ex[:, 0:nw], lg[:, 0:nw], AF.Exp,
                                             accum_out=acc[:, i, n:n + 1])
                lse_sb = hfp.tile([128, NT], F32)
                for i in range(NT):
                    tot = smallp.tile([128, 1], F32, tag="tot")
                    nc.vector.tensor_reduce(tot, acc[:, i, :], AX.X, ALU.add)
                    nc.scalar.activation(lse_sb[:, i:i + 1], tot, AF.Ln)
                nc.sync.dma_start(out=lse_d.rearrange("(i p) -> p i", p=128),
                                  in_=lse_sb)

    nc.finalize()
    return nc


def kernel(x, y, tok_emb, pos_emb, ln1_w, ln1_b, attn_w, attn_b, proj_w, proj_b,
           ln2_w, ln2_b, ffn_w1, ffn_b1, ffn_w2, ffn_b2, lnf_w, lnf_b,
           head_w, head_b):
    x = np.asarray(x)
    y = np.asarray(y)
    f = lambda a: np.asarray(a, np.float32)
    tok_emb, pos_emb = f(tok_emb), f(pos_emb)
    ln1_w, ln1_b, attn_w, attn_b = f(ln1_w), f(ln1_b), f(attn_w), f(attn_b)
    proj_w, proj_b, ln2_w, ln2_b = f(proj_w), f(proj_b), f(ln2_w), f(ln2_b)
    ffn_w1, ffn_b1, ffn_w2, ffn_b2 = f(ffn_w1), f(ffn_b1), f(ffn_w2), f(ffn_b2)
    lnf_w, lnf_b, head_w, head_b = f(lnf_w), f(lnf_b), f(head_w), f(head_b)

    bf = lambda a: np.ascontiguousarray(a.astype(ml_dtypes.bfloat16))

    # ---- host preprocessing: embeddings + LN folding + head-dim padding ----
    h0 = tok_emb[x] + pos_emb[None, :, :]                      # [B,T,C] f32

    WQKV = np.zeros((L, C, 2 * CP + C), np.float32)
    BQK = np.zeros((L, 2 * CP), np.float32)
    WPROJ = np.zeros((L, CP, C), np.float32)
    PB = np.zeros((L, C), np.float32)
    W1 = np.zeros((L, C, F), np.float32)
    B1 = np.zeros((L, F), np.float32)
    for l in range(L):
        aw = ln1_w[l][:, None] * attn_w[l]                     # [C, 3C]
        ab = attn_b[l] + ln1_b[l] @ attn_w[l]                  # [3C]
        for hh in range(H):
            WQKV[l, :, hh * 128:hh * 128 + D] = aw[:, hh * D:(hh + 1) * D]
            WQKV[l, :, CP + hh * 128:CP + hh * 128 + D] = aw[:, C + hh * D:C + (hh + 1) * D]
            BQK[l, hh * 128:hh * 128 + D] = ab[hh * D:(hh + 1) * D]
            BQK[l, CP + hh * 128:CP + hh * 128 + D] = ab[C + hh * D:C + (hh + 1) * D]
            WPROJ[l, hh * 128 + 1:hh * 128 + 1 + D, :] = proj_w[l][hh * D:(hh + 1) * D, :]
        WQKV[l, :, 2 * CP:] = aw[:, 2 * C:]                    # v (unpadded)
        PB[l] = proj_b[l] + ab[2 * C:] @ proj_w[l]             # v-bias folded
        W1[l] = ln2_w[l][:, None] * ffn_w1[l]
        B1[l] = ffn_b1[l] + ln2_b[l] @ ffn_w1[l]
    WHE = lnf_w[:, None] * head_w                              # [C, V]
    HB = head_b + lnf_b @ head_w                               # [V]

    has_qkbias = bool(np.any(BQK))
    has_pbias = bool(np.any(PB))
    has_b2bias = bool(np.any(ffn_b2))
    has_hbias = bool(np.any(HB))
    if has_hbias:
        WHEAD = np.zeros((896, V), np.float32)
        WHEAD[:C] = WHE
        WHEAD[C] = HB
    else:
        WHEAD = WHE

    debug = os.environ.get("KERNEL_DEBUG") == "1"
    key = (has_qkbias, has_pbias, has_b2bias, has_hbias, debug)
    if key not in _cache:
        _cache[key] = _build_program(*key[:4], debug=debug)
    nc = _cache[key]

    shared = {
        "wqkv": bf(WQKV), "wproj": bf(WPROJ), "w1": bf(W1), "w2": bf(ffn_w2),
        "b1": np.ascontiguousarray(B1), "whead": bf(WHEAD),
        "idn": np.eye(128, dtype=ml_dtypes.bfloat16),
        "ut": np.triu(np.ones((128, 128), ml_dtypes.bfloat16)),
    }
    if has_qkbias:
        shared["bqk"] = np.ascontiguousarray(BQK)
    if has_pbias:
        shared["pb"] = np.ascontiguousarray(PB)
    if has_b2bias:
        shared["b2"] = np.ascontiguousarray(ffn_b2)

    in_maps = [dict(shared, h0=np.ascontiguousarray(h0[c])) for c in range(B)]

    trace = os.environ.get("KERNEL_TRACE") == "1"
    res = run_bass_kernel_spmd(nc, in_maps, core_ids=list(range(NCORES)),
                               trace=trace)
    if trace and res.exec_time_ns is not None:
        print(f"HW exec time: {res.exec_time_ns} ns")
        kernel.last_exec_time_ns = res.exec_time_ns

    if debug:
        kernel.debug_out = res.results[0]
    logits = np.stack([res.results[c]["logits"] for c in range(B)])   # [B,T,V]
    lse = np.stack([res.results[c]["lse"] for c in range(B)])          # [B,T]
    ly = np.take_along_axis(logits.reshape(B * T, V),
                            y.reshape(B * T, 1).astype(np.int64), axis=1)[:, 0]
    loss = np.float32(np.mean(lse.reshape(B * T) - ly))
    return logits, loss


# revision 18
# speedup vs baseline: 1.2999x; 1.0049x over previous
"""Bass/Trainium2 kernel for nn_BigramLanguageModel (6-layer GPT, B=8,T=1024,C=768,V=32000).

Strategy: data-parallel over batch across the 8 NeuronCores (one batch element
per core, no collectives). Each core runs the full transformer + lm_head +
logsumexp for its sequence; the host folds LN weights/biases into adjacent
matmul weights, gathers embeddings, and combines per-core loss partials.
"""

import os
import numpy as np
import ml_dtypes

import concourse.bass as bass
import concourse.mybir as mybir
import concourse.tile as tile
from concourse import bacc
from concourse.bass_utils import run_bass_kernel_spmd

_orig_get_act_tables = bacc.get_activation_tables

def _steered_act_tables(arch):
    tabs = {k: set(v) for k, v in _orig_get_act_tables(arch).items()}
    combined = tabs.get("natural_log_exp_and_others")
    if combined and AF.Exp in combined and AF.Ln in combined:
        for name, fns in tabs.items():
            if name != "natural_log_exp_and_others":
                fns.discard(AF.Exp)
                fns.discard(AF.Ln)
    return tabs

bacc.get_activation_tables = _steered_act_tables

F32 = mybir.dt.float32
BF16 = mybir.dt.bfloat16
AF = mybir.ActivationFunctionType
ALU = mybir.AluOpType
AX = mybir.AxisListType

V = 32000
T = 1024
C = 768
L = 6
H = 8
B = 8
D = 96          # real head dim
DP = 128        # padded head dim
CP = H * DP     # 1024, padded attn-concat dim
F = 4 * C       # 3072
EPS = 1e-5
NCORES = 8
NT = T // 128   # 8 token tiles
NCO = C // 128  # 6 feature tiles
NV = (V + 511) // 512  # 63 vocab chunks (62 full + 1 of 256)

_cache = {}


def _ln_into(nc, pools, h_sb, hnT, idn, i):
    """LayerNorm (stats+normalize, weights pre-folded) of token tile i of h_sb
    into transposed hnT[:, :, i*128:(i+1)*128] (bf16)."""
    small = pools["small"]
    psA = pools["psA"]
    stats = small.tile([128, 3, 6], F32, tag="stats")
    for s in range(3):
        nc.vector.bn_stats(out=stats[:, s, :], in_=h_sb[:, i, s * 256:(s + 1) * 256])
    mv = small.tile([128, 2], F32, tag="mv")
    nc.vector.bn_aggr(out=mv, in_=stats)
    # rstd = exp(-0.5*ln(var+eps)); ln/exp share one ACT table set
    rstd = small.tile([128, 1], F32, tag="rstd")
    nc.scalar.activation(rstd, mv[:, 1:2], AF.Ln, bias=pools["eps"])
    nc.scalar.activation(rstd, rstd, AF.Exp, scale=-0.5)
    nm = small.tile([128, 1], F32, tag="nm")
    nc.vector.tensor_scalar(nm, mv[:, 0:1], rstd, -1.0, ALU.mult, ALU.mult)
    hn_b = small.tile([128, 768], BF16, tag="hn_b")
    nc.scalar.activation(hn_b, h_sb[:, i, :], AF.Identity, bias=nm, scale=rstd)
    pst = psA.tile([128, 768], BF16, tag="pst")
    for co in range(NCO):
        nc.tensor.transpose(pst[:, co * 128:(co + 1) * 128],
                            hn_b[:, co * 128:(co + 1) * 128], idn)
    nc.vector.tensor_copy(
        out=hnT[:, 0:NCO, i * 128:(i + 1) * 128],
        in_=pst.rearrange("p (o t) -> p o t", t=128))


def _build_program(has_qkbias, has_pbias, has_b2bias, has_hbias, debug=False):
    KH = 896 if has_hbias else 768
    KHO = KH // 128
    nc = bacc.Bacc()
    dbg = {}
    if debug:
        dbg["hnT"] = nc.declare_dram_parameter("dbg_hnT", [128, NCO, T], BF16, True)
        dbg["qkh"] = nc.declare_dram_parameter("dbg_qkh", [128, 2, T], BF16, True)
        dbg["vaug"] = nc.declare_dram_parameter("dbg_vaug", [128, NT, H, 98], BF16, True)
        dbg["pt"] = nc.declare_dram_parameter("dbg_pt", [128, T], BF16, True)
        dbg["rsc"] = nc.declare_dram_parameter("dbg_rsc", [128, T], F32, True)
        dbg["rep"] = nc.declare_dram_parameter("dbg_rep", [128, T], BF16, True)
        dbg["attnT"] = nc.declare_dram_parameter("dbg_attnT", [128, H, T], BF16, True)
        dbg["h1"] = nc.declare_dram_parameter("dbg_h1", [128, NT, C], F32, True)

    h0_d = nc.declare_dram_parameter("h0", [T, C], F32, False)
    wqkv_d = nc.declare_dram_parameter("wqkv", [L, C, 2 * CP + C], BF16, False)
    wproj_d = nc.declare_dram_parameter("wproj", [L, CP, C], BF16, False)
    w1_d = nc.declare_dram_parameter("w1", [L, C, F], BF16, False)
    w2_d = nc.declare_dram_parameter("w2", [L, F, C], BF16, False)
    b1_d = nc.declare_dram_parameter("b1", [L, F], F32, False)
    idn_d = nc.declare_dram_parameter("idn", [128, 128], BF16, False)
    ut_d = nc.declare_dram_parameter("ut", [128, 128], BF16, False)
    whead_d = nc.declare_dram_parameter("whead", [KH, V], BF16, False)
    if has_qkbias:
        bqk_d = nc.declare_dram_parameter("bqk", [L, 2 * CP], F32, False)
    if has_pbias:
        pb_d = nc.declare_dram_parameter("pb", [L, C], F32, False)
    if has_b2bias:
        b2_d = nc.declare_dram_parameter("b2", [L, C], F32, False)
    logits_d = nc.declare_dram_parameter("logits", [T, V], F32, True)
    lse_d = nc.declare_dram_parameter("lse", [T], F32, True)

    with tile.TileContext(nc) as tc:
        import contextlib
        stack = contextlib.ExitStack()
        with stack:
            const = stack.enter_context(tc.tile_pool(name="const", bufs=1))
            hnp = stack.enter_context(tc.tile_pool(name="hnp", bufs=1))
            smallp = stack.enter_context(tc.tile_pool(name="small", bufs=4))

            idn = const.tile([128, 128], BF16)
            nc.sync.dma_start(out=idn, in_=idn_d[:, :])
            ut = const.tile([128, 128], BF16)
            nc.sync.dma_start(out=ut, in_=ut_d[:, :])
            h_sb = const.tile([128, NT, C], F32)
            h0r = h0_d.rearrange("(i p) c -> p i c", p=128)
            for i in range(NT):
                nc.sync.dma_start(out=h_sb[:, i, :], in_=h0r[:, i, :])
            eps_sb = const.tile([128, 1], F32)
            nc.vector.memset(eps_sb, EPS)
            ones96 = const.tile([128, 96], BF16)
            nc.vector.memset(ones96, 1.0)

            # rolling per-layer weight pools: wqkv prefetched one layer ahead
            wq_ctx = {}
            wq_sb = {}

            def open_wqkv(l):
                ctx = tc.tile_pool(name=f"wqkv{l}", bufs=1, side="right")
                pool = ctx.__enter__()
                t = pool.tile([128, NCO, 2 * CP + C], BF16)
                wqr = wqkv_d[l].rearrange("(o p) m -> p o m", p=128)
                for co in range(NCO):
                    nc.sync.dma_start(out=t[:, co, :], in_=wqr[:, co, :])
                wq_ctx[l] = ctx
                wq_sb[l] = t

            open_wqkv(0)

            hnA = hnp.tile([128, NCO, T], BF16, tag="hnA")
            with tc.tile_pool(name="psA0", bufs=2, space="PSUM") as psA0:
                pools = {"small": smallp, "psA": psA0, "eps": eps_sb}
                for i in range(NT):
                    _ln_into(nc, pools, h_sb, hnA, idn, i)

            for l in range(L):
                wqkv_sb = wq_sb[l]
                pctx = tc.tile_pool(name=f"wproj{l}", bufs=1, side="right")
                ppool = pctx.__enter__()
                wproj_sb = ppool.tile([128, H, C], BF16)
                nc.sync.dma_start(
                    out=wproj_sb, in_=wproj_d[l].rearrange("(o p) n -> p o n", p=128))
                if has_qkbias:
                    bqk_sb = smallp.tile([128, 16], F32, tag="bqk")
                    nc.sync.dma_start(
                        out=bqk_sb, in_=bqk_d[l].rearrange("(o p) -> p o", p=128))

                hnT = hnA

                # ---- Phase B: V (token-major) + V_aug ----
                at_ctx = tc.tile_pool(name=f"atT{l}", bufs=1, side="right")
                atpool = at_ctx.__enter__()
                attnT = atpool.tile([128, H, T], BF16)
                nc.vector.memset(attnT[0:1, :, :], 0.0)
                nc.gpsimd.memset(attnT[96:128, :, :], 0.0)
                with tc.tile_pool(name=f"vau{l}", bufs=1) as vpool, \
                     tc.tile_pool(name=f"att{l}", bufs=3) as apool:
                    vaug = vpool.tile([128, NT, H, 98], BF16)
                    nc.gpsimd.memset(vaug[:, :, :, 0:1], 1.0)
                    with tc.tile_pool(name=f"psB{l}", bufs=2, space="PSUM") as psB:
                        for j in range(NT):
                            vps = psB.tile([128, C], F32, tag="vps")
                            for n0, nw in ((0, 512), (512, 256)):
                                for co in range(NCO):
                                    nc.tensor.matmul(
                                        vps[:, n0:n0 + nw],
                                        lhsT=hnT[:, co, j * 128:(j + 1) * 128],
                                        rhs=wqkv_sb[:, co, 2 * CP + n0:2 * CP + n0 + nw],
                                        start=(co == 0), stop=(co == NCO - 1))
                            nc.vector.tensor_copy(
                                out=vaug[:, j, :, 1:97],
                                in_=vps.rearrange("p (h d) -> p h d", d=96))

                    if debug and l == 0:
                        nc.sync.dma_start(out=dbg["hnT"][:, :, :], in_=hnT)
                        nc.sync.dma_start(out=dbg["vaug"][:, :, :, :], in_=vaug)
                    # ---- Phase C: attention per head ----
                    psC_ctx = tc.tile_pool(name=f"psC{l}", bufs=4, space="PSUM")
                    psC = psC_ctx.__enter__()
                    psO_ctx = tc.tile_pool(name=f"psO{l}", bufs=2, space="PSUM")
                    psO = psO_ctx.__enter__()

                    def emit_norm(hh, ops, l=l):
                        rs = apool.tile([128, T], F32, tag="rs")
                        nc.scalar.activation(rs[0:1, :], ops[0:1, :], AF.Ln)
                        rsb = apool.tile([128, T], BF16, tag="rsb")
                        nc.scalar.activation(rsb[0:1, :], rs[0:1, :], AF.Exp,
                                             scale=-1.0)
                        rep = apool.tile([128, T], BF16, tag="rep")
                        nc.gpsimd.partition_broadcast(rep[0:97, :], rsb[0:1, :])
                        nc.vector.tensor_tensor(
                            out=attnT[0:97, hh, :], in0=ops[0:97, :],
                            in1=rep[0:97, :], op=ALU.mult)
                        if debug and l == 0 and hh == 0:
                            osc = apool.tile([128, T], F32, tag="osc")
                            nc.vector.tensor_copy(out=osc, in_=ops)
                            nc.sync.dma_start(out=dbg["rsc"][:, :], in_=osc)
                            nc.sync.dma_start(out=dbg["rep"][:, :], in_=rep)

                    pending = None
                    for hh in range(H):
                        qkh = apool.tile([128, 2, T], BF16, tag="qkh")
                        for qk in range(2):
                            for n0 in (0, 512):
                                ps = psC.tile([128, 512], F32, tag="big")
                                for co in range(NCO):
                                    nc.tensor.matmul(
                                        ps,
                                        lhsT=wqkv_sb[:, co,
                                                     qk * CP + hh * 128:qk * CP + hh * 128 + 128],
                                        rhs=hnT[:, co, n0:n0 + 512],
                                        start=(co == 0), stop=(co == NCO - 1))
                                if has_qkbias:
                                    nc.scalar.activation(
                                        qkh[:, qk, n0:n0 + 512], ps, AF.Identity,
                                        bias=bqk_sb[:, qk * 8 + hh:qk * 8 + hh + 1])
                                else:
                                    nc.vector.tensor_copy(
                                        out=qkh[:, qk, n0:n0 + 512], in_=ps)

                        if pending is not None:
                            emit_norm(*pending)
                        ops = psO.tile([128, T], F32, tag="ops")
                        for j in range(NT):
                            q0 = j * 128
                            pt = apool.tile([128, T], BF16, tag="pt")
                            for c0 in range(q0 // 512 * 512, T, 512):
                                a = max(q0, c0)
                                st = psC.tile([128, 512], F32, tag="big")
                                nc.tensor.matmul(
                                    st[:, 0:c0 + 512 - a],
                                    lhsT=qkh[:, 1, q0:q0 + 128],
                                    rhs=qkh[:, 0, a:c0 + 512],
                                    start=True, stop=True)
                                nc.scalar.activation(pt[:, a:c0 + 512],
                                                     st[:, 0:c0 + 512 - a], AF.Exp,
                                                     scale=float(D) ** -0.5)
                            nc.vector.tensor_tensor(
                                out=pt[:, q0:q0 + 128], in0=pt[:, q0:q0 + 128],
                                in1=ut, op=ALU.mult)
                            if debug and l == 0 and hh == 0 and j == 0:
                                nc.sync.dma_start(out=dbg["pt"][:, :], in_=pt)
                            for c0 in range(0, T, 512):
                                if c0 + 512 <= q0:
                                    continue
                                a = max(q0, c0)
                                nc.tensor.matmul(
                                    ops[0:97, a:c0 + 512],
                                    lhsT=vaug[:, j, hh, 0:97],
                                    rhs=pt[:, a:c0 + 512],
                                    start=(j == 0),
                                    stop=(j == min(7, (c0 + 511) // 128)))
                        if debug and l == 0 and hh == 0:
                            nc.sync.dma_start(out=dbg["qkh"][:, :, :], in_=qkh)
                        pending = (hh, ops)
                    emit_norm(*pending)
                    psO_ctx.__exit__(None, None, None)
                    psC_ctx.__exit__(None, None, None)
                    if debug and l == 0:
                        nc.sync.dma_start(out=dbg["attnT"][:, :, :], in_=attnT)

                # ---- Phase D: proj + residual + LN2 ----
                hnB = hnp.tile([128, NCO, T], BF16, tag="hnB")
                with tc.tile_pool(name=f"psD{l}", bufs=2, space="PSUM") as psD, \
                     tc.tile_pool(name=f"psDE{l}", bufs=2, space="PSUM") as psDE:
                    pools = {"small": smallp, "psA": psDE, "eps": eps_sb}
                    if has_pbias:
                        pbrow = smallp.tile([128, C], F32, tag="pbrow")
                        nc.sync.dma_start(out=pbrow[0:1, :], in_=pb_d[l][None, :])
                        pbrep = smallp.tile([128, C], F32, tag="pbrep")
                        nc.gpsimd.partition_broadcast(pbrep, pbrow[0:1, :])
                    for i in range(NT):
                        pp = psD.tile([128, C], F32, tag="pp")
                        for n0, nw in ((0, 512), (512, 256)):
                            for kt in range(H):
                                nc.tensor.matmul(
                                    pp[:, n0:n0 + nw],
                                    lhsT=attnT[:, kt, i * 128:(i + 1) * 128],
                                    rhs=wproj_sb[:, kt, n0:n0 + nw],
                                    start=(kt == 0), stop=(kt == H - 1))
                        nc.vector.tensor_add(out=h_sb[:, i, :], in0=h_sb[:, i, :],
                                             in1=pp)
                        if has_pbias:
                            nc.vector.tensor_add(out=h_sb[:, i, :],
                                                 in0=h_sb[:, i, :], in1=pbrep)
                        _ln_into(nc, pools, h_sb, hnB, idn, i)
                if debug and l == 0:
                    nc.sync.dma_start(out=dbg["h1"][:, :, :], in_=h_sb)
                at_ctx.__exit__(None, None, None)
                pctx.__exit__(None, None, None)
                wq_ctx[l].__exit__(None, None, None)

                # ---- Phase F: MLP (+ next layer's LN1) ----
                wctx = tc.tile_pool(name=f"wffn{l}", bufs=1)
                wpool = wctx.__enter__()
                w1_sb = wpool.tile([128, NCO, F], BF16, tag="w1")
                nc.sync.dma_start(
                    out=w1_sb, in_=w1_d[l].rearrange("(o p) m -> p o m", p=128))
                w2_sb = wpool.tile([128, F // 128, C], BF16, tag="w2")
                nc.sync.dma_start(
                    out=w2_sb, in_=w2_d[l].rearrange("(o p) n -> p o n", p=128))
                b1_sb = smallp.tile([128, F // 128], F32, tag="b1")
                nc.sync.dma_start(
                    out=b1_sb, in_=b1_d[l].rearrange("(o p) -> p o", p=128))
                if has_b2bias:
                    b2row = smallp.tile([128, C], F32, tag="b2row")
                    nc.sync.dma_start(out=b2row[0:1, :], in_=b2_d[l][None, :])
                    b2rep = smallp.tile([128, C], F32, tag="b2rep")
                    nc.gpsimd.partition_broadcast(b2rep, b2row[0:1, :])

                if l + 1 < L:
                    hnA = hnp.tile([128, NCO, T], BF16, tag="hnA")
                with tc.tile_pool(name=f"fc{l}", bufs=1) as fcp, \
                     tc.tile_pool(name=f"psF1{l}", bufs=2, space="PSUM") as psF1, \
                     tc.tile_pool(name=f"psF2{l}", bufs=2, space="PSUM") as psF2, \
                     tc.tile_pool(name=f"psFA{l}", bufs=2, space="PSUM") as psFA:
                    pools = {"small": smallp, "psA": psFA, "eps": eps_sb}
                    for quarter in range(4):
                        t0 = quarter * 256
                        fc1 = fcp.tile([128, F // 128, 256], BF16, tag="fc1")
                        for m in range(F // 128):
                            fp = psF1.tile([128, 256], F32, tag="fp")
                            for co in range(NCO):
                                nc.tensor.matmul(
                                    fp, lhsT=w1_sb[:, co, m * 128:(m + 1) * 128],
                                    rhs=hnB[:, co, t0:t0 + 256],
                                    start=(co == 0), stop=(co == NCO - 1))
                            nc.scalar.activation(fc1[:, m, :], fp, AF.Relu,
                                                 bias=b1_sb[:, m:m + 1])
                        # prefetch next layer's qkv weights
                        if quarter == 0 and l + 1 < L:
                            open_wqkv(l + 1)
                        for ii in range(2):
                            i = quarter * 2 + ii
                            p2 = psF2.tile([128, C], F32, tag="p2")
                            for n0, nw in ((0, 512), (512, 256)):
                                for kt in range(F // 128):
                                    nc.tensor.matmul(
                                        p2[:, n0:n0 + nw],
                                        lhsT=fc1[:, kt, ii * 128:(ii + 1) * 128],
                                        rhs=w2_sb[:, kt, n0:n0 + nw],
                                        start=(kt == 0), stop=(kt == F // 128 - 1))
                            nc.vector.tensor_add(out=h_sb[:, i, :],
                                                 in0=h_sb[:, i, :], in1=p2)
                            if has_b2bias:
                                nc.vector.tensor_add(out=h_sb[:, i, :],
                                                     in0=h_sb[:, i, :], in1=b2rep)
                            if l + 1 < L:
                                _ln_into(nc, pools, h_sb, hnA, idn, i)
                wctx.__exit__(None, None, None)

            # ---- Phase G: final LN + lm_head + logsumexp ----
            hfp = stack.enter_context(tc.tile_pool(name="hfp", bufs=1))
            hfT = hfp.tile([128, KHO, T], BF16, tag="hnT2")
            with tc.tile_pool(name="psLNf", bufs=2, space="PSUM") as psA:
                pools = {"small": smallp, "psA": psA, "eps": eps_sb}
                for i in range(NT):
                    _ln_into(nc, pools, h_sb, hfT, idn, i)
            if has_hbias:
                nc.vector.memset(hfT[:, 6, :], 0.0)
                nc.vector.memset(hfT[0:1, 6, :], 1.0)

            acc = hfp.tile([128, NT, NV], F32)
            with tc.tile_pool(name="whp", bufs=3) as whp, \
                 tc.tile_pool(name="lgp", bufs=6) as lgp, \
                 tc.tile_pool(name="exg", bufs=2) as exg, \
                 tc.tile_pool(name="psG", bufs=6, space="PSUM") as psG:
                for n in range(NV):
                    n0 = n * 512
                    nw = min(512, V - n0)
                    wh = whp.tile([128, KHO, 512], BF16, tag="wh")
                    nc.sync.dma_start(
                        out=wh[:, :, 0:nw],
                        in_=whead_d[:, n0:n0 + nw].rearrange("(o p) m -> p o m", p=128))
                    for i in range(NT):
                        lp = psG.tile([128, 512], F32, tag="lp")
                        for co in range(KHO):
                            nc.tensor.matmul(
                                lp[:, 0:nw], lhsT=hfT[:, co, i * 128:(i + 1) * 128],
                                rhs=wh[:, co, 0:nw],
                                start=(co == 0), stop=(co == KHO - 1))
                        lg = lgp.tile([128, 512], F32, tag="lg")
                        nc.vector.tensor_copy(out=lg[:, 0:nw], in_=lp[:, 0:nw])
                        nc.sync.dma_start(
                            out=logits_d[i * 128:(i + 1) * 128, n0:n0 + nw],
                            in_=lg[:, 0:nw])
                        ex = exg.tile([128, 512], BF16, tag="ex")
                        nc.scalar.activation(ex[:, 0:nw], lg[:, 0:nw], AF.Exp,
                                             accum_out=acc[:, i, n:n + 1])
                lse_sb = hfp.tile([128, NT], F32)
                for i in range(NT):
                    tot = smallp.tile([128, 1], F32, tag="tot")
                    nc.vector.tensor_reduce(tot, acc[:, i, :], AX.X, ALU.add)
                    nc.scalar.activation(lse_sb[:, i:i + 1], tot, AF.Ln)
                nc.sync.dma_start(out=lse_d.rearrange("(i p) -> p i", p=128),
                                  in_=lse_sb)

    nc.finalize()
    return nc


def kernel(x, y, tok_emb, pos_emb, ln1_w, ln1_b, attn_w, attn_b, proj_w, proj_b,
           ln2_w, ln2_b, ffn_w1, ffn_b1, ffn_w2, ffn_b2, lnf_w, lnf_b,
           head_w, head_b):
    x = np.asarray(x)
    y = np.asarray(y)
    f = lambda a: np.asarray(a, np.float32)
    tok_emb, pos_emb = f(tok_emb), f(pos_emb)
    ln1_w, ln1_b, attn_w, attn_b = f(ln1_w), f(ln1_b), f(attn_w), f(attn_b)
    proj_w, proj_b, ln2_w, ln2_b = f(proj_w), f(proj_b), f(ln2_w), f(ln2_b)
    ffn_w1, ffn_b1, ffn_w2, ffn_b2 = f(ffn_w1), f(ffn_b1), f(ffn_w2), f(ffn_b2)
    lnf_w, lnf_b, head_w, head_b = f(lnf_w), f(lnf_b), f(head_w), f(head_b)

    bf = lambda a: np.ascontiguousarray(a.astype(ml_dtypes.bfloat16))

    # ---- host preprocessing: embeddings + LN folding + head-dim padding ----
    h0 = tok_emb[x] + pos_emb[None, :, :]                      # [B,T,C] f32

    WQKV = np.zeros((L, C, 2 * CP + C), np.float32)
    BQK = np.zeros((L, 2 * CP), np.float32)
    WPROJ = np.zeros((L, CP, C), np.float32)
    PB = np.zeros((L, C), np.float32)
    W1 = np.zeros((L, C, F), np.float32)
    B1 = np.zeros((L, F), np.float32)
    for l in range(L):
        aw = ln1_w[l][:, None] * attn_w[l]                     # [C, 3C]
        ab = attn_b[l] + ln1_b[l] @ attn_w[l]                  # [3C]
        for hh in range(H):
            WQKV[l, :, hh * 128:hh * 128 + D] = aw[:, hh * D:(hh + 1) * D]
            WQKV[l, :, CP + hh * 128:CP + hh * 128 + D] = aw[:, C + hh * D:C + (hh + 1) * D]
            BQK[l, hh * 128:hh * 128 + D] = ab[hh * D:(hh + 1) * D]
            BQK[l, CP + hh * 128:CP + hh * 128 + D] = ab[C + hh * D:C + (hh + 1) * D]
            WPROJ[l, hh * 128 + 1:hh * 128 + 1 + D, :] = proj_w[l][hh * D:(hh + 1) * D, :]
        WQKV[l, :, 2 * CP:] = aw[:, 2 * C:]                    # v (unpadded)
        PB[l] = proj_b[l] + ab[2 * C:] @ proj_w[l]             # v-bias folded
        W1[l] = ln2_w[l][:, None] * ffn_w1[l]
        B1[l] = ffn_b1[l] + ln2_b[l] @ ffn_w1[l]
    WHE = lnf_w[:, None] * head_w                              # [C, V]
    HB = head_b + lnf_b @ head_w                               # [V]

    has_qkbias = bool(np.any(BQK))
    has_pbias = bool(np.any(PB))
    has_b2bias = bool(np.any(ffn_b2))
    has_hbias = bool(np.any(HB))
    if has_hbias:
        WHEAD = np.zeros((896, V), np.float32)
        WHEAD[:C] = WHE
        WHEAD[C] = HB
    else:
        WHEAD = WHE

    debug = os.environ.get("KERNEL_DEBUG") == "1"
    key = (has_qkbias, has_pbias, has_b2bias, has_hbias, debug)
    if key not in _cache:
        _cache[key] = _build_program(*key[:4], debug=debug)
    nc = _cache[key]

    shared = {
        "wqkv": bf(WQKV), "wproj": bf(WPROJ), "w1": bf(W1), "w2": bf(ffn_w2),
        "b1": np.ascontiguousarray(B1), "whead": bf(WHEAD),
        "idn": np.eye(128, dtype=ml_dtypes.bfloat16),
        "ut": np.triu(np.ones((128, 128), ml_dtypes.bfloat16)),
    }
    if has_qkbias:
        shared["bqk"] = np.ascontiguousarray(BQK)
    if has_pbias:
        shared["pb"] = np.ascontiguousarray(PB)
    if has_b2bias:
        shared["b2"] = np.ascontiguousarray(ffn_b2)

    in_maps = [dict(shared, h0=np.ascontiguousarray(h0[c])) for c in range(B)]

    trace = os.environ.get("KERNEL_TRACE") == "1"
    res = run_bass_kernel_spmd(nc, in_maps, core_ids=list(range(NCORES)),
                               trace=trace)
    if trace and res.exec_time_ns is not None:
        print(f"HW exec time: {res.exec_time_ns} ns")
        kernel.last_exec_time_ns = res.exec_time_ns

    if debug:
        kernel.debug_out = res.results[0]
    logits = np.stack([res.results[c]["logits"] for c in range(B)])   # [B,T,V]
    lse = np.stack([res.results[c]["lse"] for c in range(B)])          # [B,T]
    ly = np.take_along_axis(logits.reshape(B * T, V),
                            y.reshape(B * T, 1).astype(np.int64), axis=1)[:, 0]
    loss = np.float32(np.mean(lse.reshape(B * T) - ly))
    return logits, loss


# revision 19
# speedup vs baseline: 1.3008x; 1.0007x over previous
"""Bass/Trainium2 kernel for nn_BigramLanguageModel (6-layer GPT, B=8,T=1024,C=768,V=32000).

Strategy: data-parallel over batch across the 8 NeuronCores (one batch element
per core, no collectives). Each core runs the full transformer + lm_head +
logsumexp for its sequence; the host folds LN weights/biases into adjacent
matmul weights, gathers embeddings, and combines per-core loss partials.
"""

import os
import numpy as np
import ml_dtypes

import concourse.bass as bass
import concourse.mybir as mybir
import concourse.tile as tile
from concourse import bacc
from concourse.bass_utils import run_bass_kernel_spmd

_orig_get_act_tables = bacc.get_activation_tables

def _steered_act_tables(arch):
    tabs = {k: set(v) for k, v in _orig_get_act_tables(arch).items()}
    combined = tabs.get("natural_log_exp_and_others")
    if combined and AF.Exp in combined and AF.Ln in combined:
        for name, fns in tabs.items():
            if name != "natural_log_exp_and_others":
                fns.discard(AF.Exp)
                fns.discard(AF.Ln)
    return tabs

bacc.get_activation_tables = _steered_act_tables

F32 = mybir.dt.float32
BF16 = mybir.dt.bfloat16
AF = mybir.ActivationFunctionType
ALU = mybir.AluOpType
AX = mybir.AxisListType

V = 32000
T = 1024
C = 768
L = 6
H = 8
B = 8
D = 96          # real head dim
DP = 128        # padded head dim
CP = H * DP     # 1024, padded attn-concat dim
F = 4 * C       # 3072
EPS = 1e-5
NCORES = 8
NT = T // 128   # 8 token tiles
NCO = C // 128  # 6 feature tiles
NV = (V + 511) // 512  # 63 vocab chunks (62 full + 1 of 256)

_cache = {}


def _ln_into(nc, pools, h_sb, hnT, idn, i):
    """LayerNorm (stats+normalize, weights pre-folded) of token tile i of h_sb
    into transposed hnT[:, :, i*128:(i+1)*128] (bf16)."""
    small = pools["small"]
    psA = pools["psA"]
    stats = small.tile([128, 3, 6], F32, tag="stats")
    for s in range(3):
        nc.vector.bn_stats(out=stats[:, s, :], in_=h_sb[:, i, s * 256:(s + 1) * 256])
    mv = small.tile([128, 2], F32, tag="mv")
    nc.vector.bn_aggr(out=mv, in_=stats)
    # rstd = exp(-0.5*ln(var+eps)); ln/exp share one ACT table set
    rstd = small.tile([128, 1], F32, tag="rstd")
    nc.scalar.activation(rstd, mv[:, 1:2], AF.Ln, bias=pools["eps"])
    nc.scalar.activation(rstd, rstd, AF.Exp, scale=-0.5)
    nm = small.tile([128, 1], F32, tag="nm")
    nc.vector.tensor_scalar(nm, mv[:, 0:1], rstd, -1.0, ALU.mult, ALU.mult)
    hn_b = small.tile([128, 768], BF16, tag="hn_b")
    nc.scalar.activation(hn_b, h_sb[:, i, :], AF.Identity, bias=nm, scale=rstd)
    pst = psA.tile([128, 768], BF16, tag="pst")
    for co in range(NCO):
        nc.tensor.transpose(pst[:, co * 128:(co + 1) * 128],
                            hn_b[:, co * 128:(co + 1) * 128], idn)
    nc.vector.tensor_copy(
        out=hnT[:, 0:NCO, i * 128:(i + 1) * 128],
        in_=pst.rearrange("p (o t) -> p o t", t=128))


def _build_program(has_qkbias, has_pbias, has_b2bias, has_hbias, debug=False):
    KH = 896 if has_hbias else 768
    KHO = KH // 128
    nc = bacc.Bacc()
    dbg = {}
    if debug:
        dbg["hnT"] = nc.declare_dram_parameter("dbg_hnT", [128, NCO, T], BF16, True)
        dbg["qkh"] = nc.declare_dram_parameter("dbg_qkh", [128, 2, T], BF16, True)
        dbg["vaug"] = nc.declare_dram_parameter("dbg_vaug", [128, NT, H, 98], BF16, True)
        dbg["pt"] = nc.declare_dram_parameter("dbg_pt", [128, T], BF16, True)
        dbg["rsc"] = nc.declare_dram_parameter("dbg_rsc", [128, T], F32, True)
        dbg["rep"] = nc.declare_dram_parameter("dbg_rep", [128, T], BF16, True)
        dbg["attnT"] = nc.declare_dram_parameter("dbg_attnT", [128, H, T], BF16, True)
        dbg["h1"] = nc.declare_dram_parameter("dbg_h1", [128, NT, C], F32, True)

    h0_d = nc.declare_dram_parameter("h0", [T, C], F32, False)
    wqkv_d = nc.declare_dram_parameter("wqkv", [L, C, 2 * CP + C], BF16, False)
    wproj_d = nc.declare_dram_parameter("wproj", [L, CP, C], BF16, False)
    w1_d = nc.declare_dram_parameter("w1", [L, C, F], BF16, False)
    w2_d = nc.declare_dram_parameter("w2", [L, F, C], BF16, False)
    b1_d = nc.declare_dram_parameter("b1", [L, F], F32, False)
    idn_d = nc.declare_dram_parameter("idn", [128, 128], BF16, False)
    ut_d = nc.declare_dram_parameter("ut", [128, 128], BF16, False)
    whead_d = nc.declare_dram_parameter("whead", [KH, V], BF16, False)
    if has_qkbias:
        bqk_d = nc.declare_dram_parameter("bqk", [L, 2 * CP], F32, False)
    if has_pbias:
        pb_d = nc.declare_dram_parameter("pb", [L, C], F32, False)
    if has_b2bias:
        b2_d = nc.declare_dram_parameter("b2", [L, C], F32, False)
    logits_d = nc.declare_dram_parameter("logits", [T, V], F32, True)
    lse_d = nc.declare_dram_parameter("lse", [T], F32, True)

    with tile.TileContext(nc) as tc:
        import contextlib
        stack = contextlib.ExitStack()
        with stack:
            const = stack.enter_context(tc.tile_pool(name="const", bufs=1))
            hnp = stack.enter_context(tc.tile_pool(name="hnp", bufs=1))
            smallp = stack.enter_context(tc.tile_pool(name="small", bufs=6))

            idn = const.tile([128, 128], BF16)
            nc.sync.dma_start(out=idn, in_=idn_d[:, :])
            ut = const.tile([128, 128], BF16)
            nc.sync.dma_start(out=ut, in_=ut_d[:, :])
            h_sb = const.tile([128, NT, C], F32)
            h0r = h0_d.rearrange("(i p) c -> p i c", p=128)
            for i in range(NT):
                nc.sync.dma_start(out=h_sb[:, i, :], in_=h0r[:, i, :])
            eps_sb = const.tile([128, 1], F32)
            nc.vector.memset(eps_sb, EPS)
            ones96 = const.tile([128, 96], BF16)
            nc.vector.memset(ones96, 1.0)

            # rolling per-layer weight pools: wqkv prefetched one layer ahead
            wq_ctx = {}
            wq_sb = {}

            def open_wqkv(l):
                ctx = tc.tile_pool(name=f"wqkv{l}", bufs=1, side="right")
                pool = ctx.__enter__()
                t = pool.tile([128, NCO, 2 * CP + C], BF16)
                wqr = wqkv_d[l].rearrange("(o p) m -> p o m", p=128)
                for co in range(NCO):
                    nc.sync.dma_start(out=t[:, co, :], in_=wqr[:, co, :])
                wq_ctx[l] = ctx
                wq_sb[l] = t

            open_wqkv(0)

            hnA = hnp.tile([128, NCO, T], BF16, tag="hnA")
            with tc.tile_pool(name="psA0", bufs=2, space="PSUM") as psA0:
                pools = {"small": smallp, "psA": psA0, "eps": eps_sb}
                for i in range(NT):
                    _ln_into(nc, pools, h_sb, hnA, idn, i)

            for l in range(L):
                wqkv_sb = wq_sb[l]
                pctx = tc.tile_pool(name=f"wproj{l}", bufs=1, side="right")
                ppool = pctx.__enter__()
                wproj_sb = ppool.tile([128, H, C], BF16)
                nc.sync.dma_start(
                    out=wproj_sb, in_=wproj_d[l].rearrange("(o p) n -> p o n", p=128))
                if has_qkbias:
                    bqk_sb = smallp.tile([128, 16], F32, tag="bqk")
                    nc.sync.dma_start(
                        out=bqk_sb, in_=bqk_d[l].rearrange("(o p) -> p o", p=128))

                hnT = hnA

                # ---- Phase B: V (token-major) + V_aug ----
                at_ctx = tc.tile_pool(name=f"atT{l}", bufs=1, side="right")
                atpool = at_ctx.__enter__()
                attnT = atpool.tile([128, H, T], BF16)
                nc.vector.memset(attnT[0:1, :, :], 0.0)
                nc.gpsimd.memset(attnT[96:128, :, :], 0.0)
                with tc.tile_pool(name=f"vau{l}", bufs=1) as vpool, \
                     tc.tile_pool(name=f"att{l}", bufs=3) as apool:
                    vaug = vpool.tile([128, NT, H, 98], BF16)
                    nc.gpsimd.memset(vaug[:, :, :, 0:1], 1.0)
                    with tc.tile_pool(name=f"psB{l}", bufs=2, space="PSUM") as psB:
                        for j in range(NT):
                            vps = psB.tile([128, C], F32, tag="vps")
                            for n0, nw in ((0, 512), (512, 256)):
                                for co in range(NCO):
                                    nc.tensor.matmul(
                                        vps[:, n0:n0 + nw],
                                        lhsT=hnT[:, co, j * 128:(j + 1) * 128],
                                        rhs=wqkv_sb[:, co, 2 * CP + n0:2 * CP + n0 + nw],
                                        start=(co == 0), stop=(co == NCO - 1))
                            nc.vector.tensor_copy(
                                out=vaug[:, j, :, 1:97],
                                in_=vps.rearrange("p (h d) -> p h d", d=96))

                    if debug and l == 0:
                        nc.sync.dma_start(out=dbg["hnT"][:, :, :], in_=hnT)
                        nc.sync.dma_start(out=dbg["vaug"][:, :, :, :], in_=vaug)
                    # ---- Phase C: attention per head ----
                    psC_ctx = tc.tile_pool(name=f"psC{l}", bufs=4, space="PSUM")
                    psC = psC_ctx.__enter__()
                    psO_ctx = tc.tile_pool(name=f"psO{l}", bufs=2, space="PSUM")
                    psO = psO_ctx.__enter__()

                    def emit_norm(hh, ops, l=l):
                        rs = apool.tile([128, T], F32, tag="rs")
                        nc.scalar.activation(rs[0:1, :], ops[0:1, :], AF.Ln)
                        rsb = apool.tile([128, T], BF16, tag="rsb")
                        nc.scalar.activation(rsb[0:1, :], rs[0:1, :], AF.Exp,
                                             scale=-1.0)
                        rep = apool.tile([128, T], BF16, tag="rep")
                        nc.gpsimd.partition_broadcast(rep[0:97, :], rsb[0:1, :])
                        nc.vector.tensor_tensor(
                            out=attnT[0:97, hh, :], in0=ops[0:97, :],
                            in1=rep[0:97, :], op=ALU.mult)
                        if debug and l == 0 and hh == 0:
                            osc = apool.tile([128, T], F32, tag="osc")
                            nc.vector.tensor_copy(out=osc, in_=ops)
                            nc.sync.dma_start(out=dbg["rsc"][:, :], in_=osc)
                            nc.sync.dma_start(out=dbg["rep"][:, :], in_=rep)

                    pending = None
                    for hh in range(H):
                        qkh = apool.tile([128, 2, T], BF16, tag="qkh")
                        for qk in range(2):
                            for n0 in (0, 512):
                                ps = psC.tile([128, 512], F32, tag="big")
                                for co in range(NCO):
                                    nc.tensor.matmul(
                                        ps,
                                        lhsT=wqkv_sb[:, co,
                                                     qk * CP + hh * 128:qk * CP + hh * 128 + 128],
                                        rhs=hnT[:, co, n0:n0 + 512],
                                        start=(co == 0), stop=(co == NCO - 1))
                                if has_qkbias:
                                    nc.scalar.activation(
                                        qkh[:, qk, n0:n0 + 512], ps, AF.Identity,
                                        bias=bqk_sb[:, qk * 8 + hh:qk * 8 + hh + 1])
                                else:
                                    nc.vector.tensor_copy(
                                        out=qkh[:, qk, n0:n0 + 512], in_=ps)

                        if pending is not None:
                            emit_norm(*pending)
                        ops = psO.tile([128, T], F32, tag="ops")
                        for j in range(NT):
                            q0 = j * 128
                            pt = apool.tile([128, T], BF16, tag="pt")
                            for c0 in range(q0 // 512 * 512, T, 512):
                                a = max(q0, c0)
                                st = psC.tile([128, 512], F32, tag="big")
                                nc.tensor.matmul(
                                    st[:, 0:c0 + 512 - a],
                                    lhsT=qkh[:, 1, q0:q0 + 128],
                                    rhs=qkh[:, 0, a:c0 + 512],
                                    start=True, stop=True)
                                nc.scalar.activation(pt[:, a:c0 + 512],
                                                     st[:, 0:c0 + 512 - a], AF.Exp,
                                                     scale=float(D) ** -0.5)
                            nc.vector.tensor_tensor(
                                out=pt[:, q0:q0 + 128], in0=pt[:, q0:q0 + 128],
                                in1=ut, op=ALU.mult)
                            if debug and l == 0 and hh == 0 and j == 0:
                                nc.sync.dma_start(out=dbg["pt"][:, :], in_=pt)
                            for c0 in range(0, T, 512):
                                if c0 + 512 <= q0:
                                    continue
                                a = max(q0, c0)
                                nc.tensor.matmul(
                                    ops[0:97, a:c0 + 512],
                                    lhsT=vaug[:, j, hh, 0:97],
                                    rhs=pt[:, a:c0 + 512],
                                    start=(j == 0),
                                    stop=(j == min(7, (c0 + 511) // 128)))
                        if debug and l == 0 and hh == 0:
                            nc.sync.dma_start(out=dbg["qkh"][:, :, :], in_=qkh)
                        pending = (hh, ops)
                    emit_norm(*pending)
                    psO_ctx.__exit__(None, None, None)
                    psC_ctx.__exit__(None, None, None)
                    if debug and l == 0:
                        nc.sync.dma_start(out=dbg["attnT"][:, :, :], in_=attnT)

                # ---- Phase D: proj + residual + LN2 ----
                hnB = hnp.tile([128, NCO, T], BF16, tag="hnB")
                with tc.tile_pool(name=f"psD{l}", bufs=2, space="PSUM") as psD, \
                     tc.tile_pool(name=f"psDE{l}", bufs=2, space="PSUM") as psDE:
                    pools = {"small": smallp, "psA": psDE, "eps": eps_sb}
                    if has_pbias:
                        pbrow = smallp.tile([128, C], F32, tag="pbrow")
                        nc.sync.dma_start(out=pbrow[0:1, :], in_=pb_d[l][None, :])
                        pbrep = smallp.tile([128, C], F32, tag="pbrep")
                        nc.gpsimd.partition_broadcast(pbrep, pbrow[0:1, :])
                    for i in range(NT):
                        pp = psD.tile([128, C], F32, tag="pp")
                        for n0, nw in ((0, 512), (512, 256)):
                            for kt in range(H):
                                nc.tensor.matmul(
                                    pp[:, n0:n0 + nw],
                                    lhsT=attnT[:, kt, i * 128:(i + 1) * 128],
                                    rhs=wproj_sb[:, kt, n0:n0 + nw],
                                    start=(kt == 0), stop=(kt == H - 1))
                        nc.vector.tensor_add(out=h_sb[:, i, :], in0=h_sb[:, i, :],
                                             in1=pp)
                        if has_pbias:
                            nc.vector.tensor_add(out=h_sb[:, i, :],
                                                 in0=h_sb[:, i, :], in1=pbrep)
                        _ln_into(nc, pools, h_sb, hnB, idn, i)
                if debug and l == 0:
                    nc.sync.dma_start(out=dbg["h1"][:, :, :], in_=h_sb)
                at_ctx.__exit__(None, None, None)
                pctx.__exit__(None, None, None)
                wq_ctx[l].__exit__(None, None, None)

                # ---- Phase F: MLP (+ next layer's LN1) ----
                wctx = tc.tile_pool(name=f"wffn{l}", bufs=1)
                wpool = wctx.__enter__()
                w1_sb = wpool.tile([128, NCO, F], BF16, tag="w1")
                nc.sync.dma_start(
                    out=w1_sb, in_=w1_d[l].rearrange("(o p) m -> p o m", p=128))
                w2_sb = wpool.tile([128, F // 128, C], BF16, tag="w2")
                nc.sync.dma_start(
                    out=w2_sb, in_=w2_d[l].rearrange("(o p) n -> p o n", p=128))
                b1_sb = smallp.tile([128, F // 128], F32, tag="b1")
                nc.sync.dma_start(
                    out=b1_sb, in_=b1_d[l].rearrange("(o p) -> p o", p=128))
                if has_b2bias:
                    b2row = smallp.tile([128, C], F32, tag="b2row")
                    nc.sync.dma_start(out=b2row[0:1, :], in_=b2_d[l][None, :])
                    b2rep = smallp.tile([128, C], F32, tag="b2rep")
                    nc.gpsimd.partition_broadcast(b2rep, b2row[0:1, :])

                if l + 1 < L:
                    hnA = hnp.tile([128, NCO, T], BF16, tag="hnA")
                with tc.tile_pool(name=f"fc{l}", bufs=1) as fcp, \
                     tc.tile_pool(name=f"psF1{l}", bufs=2, space="PSUM") as psF1, \
                     tc.tile_pool(name=f"psF2{l}", bufs=2, space="PSUM") as psF2, \
                     tc.tile_pool(name=f"psFA{l}", bufs=2, space="PSUM") as psFA:
                    pools = {"small": smallp, "psA": psFA, "eps": eps_sb}
                    for quarter in range(4):
                        t0 = quarter * 256
                        fc1 = fcp.tile([128, F // 128, 256], BF16, tag="fc1")
                        for m in range(F // 128):
                            fp = psF1.tile([128, 256], F32, tag="fp")
                            for co in range(NCO):
                                nc.tensor.matmul(
                                    fp, lhsT=w1_sb[:, co, m * 128:(m + 1) * 128],
                                    rhs=hnB[:, co, t0:t0 + 256],
                                    start=(co == 0), stop=(co == NCO - 1))
                            nc.scalar.activation(fc1[:, m, :], fp, AF.Relu,
                                                 bias=b1_sb[:, m:m + 1])
                        # prefetch next layer's qkv weights
                        if quarter == 0 and l + 1 < L:
                            open_wqkv(l + 1)
                        for ii in range(2):
                            i = quarter * 2 + ii
                            p2 = psF2.tile([128, C], F32, tag="p2")
                            for n0, nw in ((0, 512), (512, 256)):
                                for kt in range(F // 128):
                                    nc.tensor.matmul(
                                        p2[:, n0:n0 + nw],
                                        lhsT=fc1[:, kt, ii * 128:(ii + 1) * 128],
                                        rhs=w2_sb[:, kt, n0:n0 + nw],
                                        start=(kt == 0), stop=(kt == F // 128 - 1))
                            nc.vector.tensor_add(out=h_sb[:, i, :],
                                                 in0=h_sb[:, i, :], in1=p2)
                            if has_b2bias:
                                nc.vector.tensor_add(out=h_sb[:, i, :],
                                                     in0=h_sb[:, i, :], in1=b2rep)
                            if l + 1 < L:
                                _ln_into(nc, pools, h_sb, hnA, idn, i)
                wctx.__exit__(None, None, None)

            # ---- Phase G: final LN + lm_head + logsumexp ----
            hfp = stack.enter_context(tc.tile_pool(name="hfp", bufs=1))
            hfT = hfp.tile([128, KHO, T], BF16, tag="hnT2")
            with tc.tile_pool(name="psLNf", bufs=2, space="PSUM") as psA:
                pools = {"small": smallp, "psA": psA, "eps": eps_sb}
                for i in range(NT):
                    _ln_into(nc, pools, h_sb, hfT, idn, i)
            if has_hbias:
                nc.vector.memset(hfT[:, 6, :], 0.0)
                nc.vector.memset(hfT[0:1, 6, :], 1.0)

            acc = hfp.tile([128, NT, NV], F32)
            with tc.tile_pool(name="whp", bufs=3) as whp, \
                 tc.tile_pool(name="lgp", bufs=6) as lgp, \
                 tc.tile_pool(name="exg", bufs=2) as exg, \
                 tc.tile_pool(name="psG", bufs=6, space="PSUM") as psG:
                for n in range(NV):
                    n0 = n * 512
                    nw = min(512, V - n0)
                    wh = whp.tile([128, KHO, 512], BF16, tag="wh")
                    nc.sync.dma_start(
                        out=wh[:, :, 0:nw],
                        in_=whead_d[:, n0:n0 + nw].rearrange("(o p) m -> p o m", p=128))
                    for i in range(NT):
                        lp = psG.tile([128, 512], F32, tag="lp")
                        for co in range(KHO):
                            nc.tensor.matmul(
                                lp[:, 0:nw], lhsT=hfT[:, co, i * 128:(i + 1) * 128],
                                rhs=wh[:, co, 0:nw],
                                start=(co == 0), stop=(co == KHO - 1))
                        lg = lgp.tile([128, 512], F32, tag="lg")
                        nc.vector.tensor_copy(out=lg[:, 0:nw], in_=lp[:, 0:nw])
                        nc.sync.dma_start(
                            out=logits_d[i * 128:(i + 1) * 128, n0:n0 + nw],
                            in_=lg[:, 0:nw])
                        ex = exg.tile([128, 512], BF16, tag="ex")
                        nc.scalar.activation(ex[:, 0:nw], lg[:, 0:nw], AF.Exp,
                                             accum_out=acc[:, i, n:n + 1])
                lse_sb = hfp.tile([128, NT], F32)
                for i in range(NT):
                    tot = smallp.tile([128, 1], F32, tag="tot")
                    nc.vector.tensor_reduce(tot, acc[:, i, :], AX.X, ALU.add)
                    nc.scalar.activation(lse_sb[:, i:i + 1], tot, AF.Ln)
                nc.sync.dma_start(out=lse_d.rearrange("(i p) -> p i", p=128),
                                  in_=lse_sb)

    nc.finalize()
    return nc


def kernel(x, y, tok_emb, pos_emb, ln1_w, ln1_b, attn_w, attn_b, proj_w, proj_b,
           ln2_w, ln2_b, ffn_w1, ffn_b1, ffn_w2, ffn_b2, lnf_w, lnf_b,
           head_w, head_b):
    x = np.asarray(x)
    y = np.asarray(y)
    f = lambda a: np.asarray(a, np.float32)
    tok_emb, pos_emb = f(tok_emb), f(pos_emb)
    ln1_w, ln1_b, attn_w, attn_b = f(ln1_w), f(ln1_b), f(attn_w), f(attn_b)
    proj_w, proj_b, ln2_w, ln2_b = f(proj_w), f(proj_b), f(ln2_w), f(ln2_b)
    ffn_w1, ffn_b1, ffn_w2, ffn_b2 = f(ffn_w1), f(ffn_b1), f(ffn_w2), f(ffn_b2)
    lnf_w, lnf_b, head_w, head_b = f(lnf_w), f(lnf_b), f(head_w), f(head_b)

    bf = lambda a: np.ascontiguousarray(a.astype(ml_dtypes.bfloat16))

    # ---- host preprocessing: embeddings + LN folding + head-dim padding ----
    h0 = tok_emb[x] + pos_emb[None, :, :]                      # [B,T,C] f32

    WQKV = np.zeros((L, C, 2 * CP + C), np.float32)
    BQK = np.zeros((L, 2 * CP), np.float32)
    WPROJ = np.zeros((L, CP, C), np.float32)
    PB = np.zeros((L, C), np.float32)
    W1 = np.zeros((L, C, F), np.float32)
    B1 = np.zeros((L, F), np.float32)
    for l in range(L):
        aw = ln1_w[l][:, None] * attn_w[l]                     # [C, 3C]
        ab = attn_b[l] + ln1_b[l] @ attn_w[l]                  # [3C]
        for hh in range(H):
            WQKV[l, :, hh * 128:hh * 128 + D] = aw[:, hh * D:(hh + 1) * D]
            WQKV[l, :, CP + hh * 128:CP + hh * 128 + D] = aw[:, C + hh * D:C + (hh + 1) * D]
            BQK[l, hh * 128:hh * 128 + D] = ab[hh * D:(hh + 1) * D]
            BQK[l, CP + hh * 128:CP + hh * 128 + D] = ab[C + hh * D:C + (hh + 1) * D]
            WPROJ[l, hh * 128 + 1:hh * 128 + 1 + D, :] = proj_w[l][hh * D:(hh + 1) * D, :]
        WQKV[l, :, 2 * CP:] = aw[:, 2 * C:]                    # v (unpadded)
        PB[l] = proj_b[l] + ab[2 * C:] @ proj_w[l]             # v-bias folded
        W1[l] = ln2_w[l][:, None] * ffn_w1[l]
        B1[l] = ffn_b1[l] + ln2_b[l] @ ffn_w1[l]
    WHE = lnf_w[:, None] * head_w                              # [C, V]
    HB = head_b + lnf_b @ head_w                               # [V]

    has_qkbias = bool(np.any(BQK))
    has_pbias = bool(np.any(PB))
    has_b2bias = bool(np.any(ffn_b2))
    has_hbias = bool(np.any(HB))
    if has_hbias:
        WHEAD = np.zeros((896, V), np.float32)
        WHEAD[:C] = WHE
        WHEAD[C] = HB
    else:
        WHEAD = WHE

    debug = os.environ.get("KERNEL_DEBUG") == "1"
    key = (has_qkbias, has_pbias, has_b2bias, has_hbias, debug)
    if key not in _cache:
        _cache[key] = _build_program(*key[:4], debug=debug)
    nc = _cache[key]

    shared = {
        "wqkv": bf(WQKV), "wproj": bf(WPROJ), "w1": bf(W1), "w2": bf(ffn_w2),
        "b1": np.ascontiguousarray(B1), "whead": bf(WHEAD),
        "idn": np.eye(128, dtype=ml_dtypes.bfloat16),
        "ut": np.triu(np.ones((128, 128), ml_dtypes.bfloat16)),
    }
    if has_qkbias:
        shared["bqk"] = np.ascontiguousarray(BQK)
    if has_pbias:
        shared["pb"] = np.ascontiguousarray(PB)
    if has_b2bias:
        shared["b2"] = np.ascontiguousarray(ffn_b2)

    in_maps = [dict(shared, h0=np.ascontiguousarray(h0[c])) for c in range(B)]

    trace = os.environ.get("KERNEL_TRACE") == "1"
    res = run_bass_kernel_spmd(nc, in_maps, core_ids=list(range(NCORES)),
                               trace=trace)
    if trace and res.exec_time_ns is not None:
        print(f"HW exec time: {res.exec_time_ns} ns")
        kernel.last_exec_time_ns = res.exec_time_ns

    if debug:
        kernel.debug_out = res.results[0]
    logits = np.stack([res.results[c]["logits"] for c in range(B)])   # [B,T,V]
    lse = np.stack([res.results[c]["lse"] for c in range(B)])          # [B,T]
    ly = np.take_along_axis(logits.reshape(B * T, V),
                            y.reshape(B * T, 1).astype(np.int64), axis=1)[:, 0]
    loss = np.float32(np.mean(lse.reshape(B * T) - ly))
    return logits, loss
